# revision 13
# baseline (speedup 1.0000x reference)
"""MultiHeadedAttention block (B=4, S=2048, D=1024, H=16) on 8 TRN2 cores.

Sharding: core c handles batch b=c//2 and query-row half c%2 (1024 rows).
Each core computes full K/V projections for its batch (2x redundant within a
batch pair), attention for all 16 heads over its 1024 query rows, then
O-projection + residual + LayerNorm. No collectives.

The real-HW bottleneck is the Activation engine's exp throughput
(~0.9 ns/elem, 33.5M softmax logits per core ~ 240 us). The attention inner
loop is software-pipelined in emission order so ACT never waits:
PE computes scores(k+1) while ACT does exp(k) while PE finishes PV(k-1),
with double-buffered score PSUM and a 4-deep exp-tile ring. K projection for
the next head pair is interleaved into the PE slack of the current one.

Device layouts (per core):
  Q^T  [o=1024, r=1024]  feature-major (partitions = features), per-ot tiles
  K^T  [o, k] projected per head pair inside the attention loop (no spill)
  V    [k=2048, o=1024]  row-major per-rt tiles, with a ones column per head
  scores computed transposed: S_t[k, q] = K_h^T Q_h  (softmax along k =
  partitions; exp without max-subtraction is safe: |logits| < ~3).
  P@V with the ones-augmented V gives the softmax denominator as row DK;
  normalization multiplies by a DMA-broadcast reciprocal.
Bias algebra: bk is dropped entirely (it shifts every logit of a softmax row
equally -> exactly cancels); bv and bo are folded into the residual tensor on
the host (exact by linearity of the O projection).
"""

import sys

if "/opt/trn_rl_repo" not in sys.path:
    sys.path.insert(0, "/opt/trn_rl_repo")

import ml_dtypes
import numpy as np

import concourse.bass as bass
import concourse.mybir as mybir
import concourse.tile as tile
from concourse.bass_utils import run_bass_kernel_spmd

B, S, D, H, DK = 4, 2048, 1024, 16, 64
P = 128
M = S // 2          # query rows per core
NDT = D // P        # 8 contraction chunks
NOT = D // P        # 8 output-feature chunks (= head pairs)
NHP = H // 2        # 8 head pairs
NKT = S // P        # 16 key chunks
NQT = M // 512      # 2 query 512-chunks
NRT_K = S // 512    # 4 key-row 512-chunks
NRT_V = S // P      # 16 V row chunks
NRT_O = M // P      # 8 output row chunks
F32 = mybir.dt.float32
MM_DT = mybir.dt.float32r
AF = mybir.ActivationFunctionType
ALU = mybir.AluOpType
BF16 = mybir.dt.bfloat16


def _split_sync_waits(nc, max_waits=1):
    """Split instructions carrying more than max_waits sem waits.

    The container's walrus rejects instructions with multiple sync wait
    commands, so excess waits move onto NoOp instructions inserted just
    before, on the same engine.
    """
    idx = 0
    for f in nc.m.functions:
        for blk in f.blocks:
            newl = []
            for inst in blk.instructions:
                si = inst.sync_info
                waits = list(si.on_wait) if si is not None and si.on_wait else []
                if len(waits) > max_waits:
                    extra = waits[max_waits:]
                    si.on_wait = waits[:max_waits]
                    for j in range(0, len(extra), max_waits):
                        nop = mybir.InstNoOp(name=f"I-wsplit-{idx}", ins=[], outs=[])
                        idx += 1
                        nop.engine = inst.engine
                        nop.sync_info = mybir.SyncInfo(
                            on_wait=extra[j : j + max_waits], on_update=[]
                        )
                        newl.append(nop)
                newl.append(inst)
            blk.instructions = newl


def build_nc(loops=0, unroll=1, ab=()):
    nc = bass.Bass()
    xqT = nc.dram_tensor("xqT", [D, M], BF16, kind="ExternalInput")
    xkT = nc.dram_tensor("xkT", [D, S], BF16, kind="ExternalInput")
    xvT = nc.dram_tensor("xvT", [D, S], BF16, kind="ExternalInput")
    qres = nc.dram_tensor("qres", [M, D], F32, kind="ExternalInput")
    WqT = nc.dram_tensor("WqT", [D, D], BF16, kind="ExternalInput")
    WkT = nc.dram_tensor("WkT", [D, D], BF16, kind="ExternalInput")
    WvT = nc.dram_tensor("WvT", [D, D], BF16, kind="ExternalInput")
    WoT = nc.dram_tensor("WoT", [D, D], BF16, kind="ExternalInput")
    bqv = nc.dram_tensor("bq", [D], F32, kind="ExternalInput")
    gv = nc.dram_tensor("ln_g", [D], F32, kind="ExternalInput")
    bv2 = nc.dram_tensor("ln_b", [D], F32, kind="ExternalInput")
    onesv = nc.dram_tensor("onesv", [P, NRT_V * H], BF16, kind="ExternalInput")
    onesf = nc.dram_tensor("onesf", [DK], F32, kind="ExternalInput")
    out = nc.dram_tensor("out", [M, D], F32, kind="ExternalOutput")

    WqT_r = WqT[:, :].rearrange("(a p) o -> p a o", p=P)
    WkT_r = WkT[:, :].rearrange("(a p) o -> p a o", p=P)
    WvT_r = WvT[:, :].rearrange("(a p) o -> p a o", p=P)
    WoT_r = WoT[:, :].rearrange("(a p) o -> p a o", p=P)
    xqT_r = xqT[:, :].rearrange("(a p) r -> p a r", p=P)
    xkT_r = xkT[:, :].rearrange("(a p) r -> p a r", p=P)
    xvT_r = xvT[:, :].rearrange("(a p) r -> p a r", p=P)

    with tile.TileContext(nc) as tc:
      for _rep in range(max(1, unroll)):
        pxo_cm = tc.tile_pool(name="pxo", bufs=1)
        pxo = pxo_cm.__enter__()
        with (
            tc.tile_pool(name="pqv", bufs=1) as pqv,
        ):
            XO = [
                pxo.tile([P, M], BF16, tag=f"XO{i}", name=f"XO{i}")
                for i in range(NHP)
            ]
            if "noattn" in ab:
                for t in XO:
                    nc.vector.memset(t, 0.001)
            exf = None
            if "noexp" in ab:
                exf = [
                    pxo.tile([P, 512], BF16, tag=f"exf{h}", name=f"exf{h}")
                    for h in range(2)
                ]
                for t in exf:
                    nc.vector.memset(t, 0.001)

            QT = []
            for ot in range(NOT):
                t = pqv.tile([P, M], BF16, tag=f"QT{ot}", name=f"QT{ot}")
                QT.append(t)
            bq_p = pqv.tile([P, NOT], F32)
            nc.gpsimd.dma_start(bq_p, bqv[:].rearrange("(a p) -> p a", p=P))
            ones_t = pqv.tile([1, DK], MM_DT)
            nc.gpsimd.dma_start(
                ones_t, onesf[:].partition_broadcast(1).bitcast(MM_DT)
            )
            Vt = []
            for rt in range(NRT_V):
                t = pqv.tile([P, H, DK + 1], BF16, tag=f"Vt{rt}", name=f"Vt{rt}")
                nc.gpsimd.dma_start(
                    t[:, :, DK : DK + 1],
                    onesv[:, rt * H : (rt + 1) * H],
                )
                Vt.append(t)
            # wv loads early so phase B starts without a DMA stall
            pwv_cm = tc.tile_pool(name="pwv", bufs=NDT, side="right")
            pwv = pwv_cm.__enter__()
            wv = []
            for dt in range(NDT):
                w_t = pwv.tile([P, D], BF16, tag="wv", name=f"wv{dt}")
                nc.gpsimd.dma_start(w_t, WvT_r[:, dt, :])
                wv.append(w_t)

            pbx_cm = tc.tile_pool(name="pbx", bufs=3, side="right")
            pbx = pbx_cm.__enter__()
            psAB_cm = tc.tile_pool(name="psAB", bufs=8, space="PSUM")
            psAB = psAB_cm.__enter__()

            # ---- Phase A: Q^T = (Wq/8) @ x_q^T + bq/8, layout [o, r]
            with (
                tc.tile_pool(name="pa", bufs=NDT) as pa,
            ):
                wq = []
                xq = []
                xv_pre = {}
                for dt in range(NDT):
                    w_t = pa.tile([P, D], BF16, tag="wq", name=f"wq{dt}")
                    nc.sync.dma_start(w_t, WqT_r[:, dt, :])
                    wq.append(w_t)
                    x_t = pa.tile([P, M], BF16, tag="xq", name=f"xq{dt}")
                    nc.sync.dma_start(x_t, xqT_r[:, dt, :])
                    xq.append(x_t)
                    if dt in (2, 4, 6):
                        rt = dt // 2 - 1
                        xv_t = pbx.tile(
                            [P, NDT, P], BF16, tag="xv", name="xv"
                        )
                        nc.sync.dma_start(
                            xv_t, xvT_r[:, :, rt * P : (rt + 1) * P]
                        )
                        xv_pre[rt] = xv_t
                for ot in range(NOT):
                    for qt in range(NQT):
                        ps = psAB.tile([P, 512], F32, tag='ps', name='ps')
                        for dt in range(NDT):
                            nc.tensor.matmul(
                                ps,
                                wq[dt][:, ot * P : (ot + 1) * P],
                                xq[dt][:, qt * 512 : (qt + 1) * 512],
                                start=(dt == 0),
                                stop=(dt == NDT - 1),
                            )
                        nc.vector.tensor_scalar_add(
                            QT[ot][:, qt * 512 : (qt + 1) * 512],
                            ps,
                            bq_p[:, ot : ot + 1],
                        )

            # xk loads during phase B so phase D starts without a DMA stall
            pdx_cm = tc.tile_pool(name="pdx", bufs=NDT)
            pdx = pdx_cm.__enter__()
            xk = []
            for dt in range(NDT):
                x_t = pdx.tile([P, S], BF16, tag="xk", name=f"xk{dt}")
                nc.gpsimd.dma_start(x_t, xkT_r[:, dt, :])
                xk.append(x_t)

            # ---- Phase B: V = x_v @ Wv^T (bias folded into qres), [r, o]
            for rt in range(NRT_V):
                if rt in xv_pre:
                    xv = xv_pre.pop(rt)
                else:
                    xv = pbx.tile(
                        [P, NDT, P], BF16, tag="xv", name="xv"
                    )
                    veng = nc.sync if rt < 6 else nc.gpsimd
                    veng.dma_start(xv, xvT_r[:, :, rt * P : (rt + 1) * P])
                for o2 in range(2):
                    ps = psAB.tile([P, 512], F32, tag='ps', name='ps')
                    for dt in range(NDT):
                        nc.tensor.matmul(
                            ps,
                            xv[:, dt, :],
                            wv[dt][:, o2 * 512 : (o2 + 1) * 512],
                            start=(dt == 0),
                            stop=(dt == NDT - 1),
                        )
                    nc.vector.tensor_copy(
                        Vt[rt][:, o2 * 8 : (o2 + 1) * 8, 0:DK],
                        ps[:, :].rearrange("p (h e) -> p h e", e=DK),
                    )

            pbx_cm.__exit__(None, None, None)
            pwv_cm.__exit__(None, None, None)
            psAB_cm.__exit__(None, None, None)

            # wo prefetch during D so phase E starts without a DMA stall
            pwo_cm = tc.tile_pool(name="pwo", bufs=NDT, side="right")
            pwo = pwo_cm.__enter__()
            wo = []
            for dt in range(NDT):
                w_t = pwo.tile([P, D], BF16, tag="wo", name=f"wo{dt}")
                nc.gpsimd.dma_start(w_t, WoT_r[:, dt, :])
                wo.append(w_t)

            # ---- Phase D: K^T projection fused with attention, per head pair.
            # Software-pipelined: PE scores(k+1) || ACT exp(k) || PE pv(k-1).
            with (
                tc.tile_pool(name="pdw", bufs=2) as pdw,
                tc.tile_pool(name="pdkt", bufs=2) as pdkt,
                tc.tile_pool(name="pde", bufs=4) as pde,
                tc.tile_pool(name="pdr", bufs=2) as pdr,
                tc.tile_pool(name="psS", bufs=2, space="PSUM") as psS,
                tc.tile_pool(name="psK", bufs=1, space="PSUM") as psK,
                tc.tile_pool(name="psR", bufs=1, space="PSUM") as psR,
                tc.tile_pool(name="psPV", bufs=1, space="PSUM") as psPV,
            ):
                kts = {}
                wks = {}

                def kproj_start(hp):
                    wk = pdw.tile([P, NDT, P], BF16, tag="wk", name="wk")
                    nc.sync.dma_start(wk, WkT_r[:, :, hp * P : (hp + 1) * P])
                    wks[hp] = wk
                    kts[hp] = pdkt.tile([P, S], BF16, tag="kt", name="kt")

                def kproj_chunk(hp, rt):
                    wk = wks[hp]
                    kt_t = kts[hp]
                    ps = psK.tile([P, 512], F32, tag="kps", name="kps")
                    for dt in range(NDT):
                        nc.tensor.matmul(
                            ps,
                            wk[:, dt, :],
                            xk[dt][:, rt * 512 : (rt + 1) * 512],
                            start=(dt == 0),
                            stop=(dt == NDT - 1),
                        )
                    nc.vector.tensor_copy(
                        kt_t[:, rt * 512 : (rt + 1) * 512], ps
                    )

                def attn(hp):
                    kt_t = kts.pop(hp)
                    wks.pop(hp, None)
                    xo_t = XO[hp]
                    first_qt0 = hp == 0
                    for qt in range(NQT):
                        pv = [
                            psPV.tile(
                                [DK + 1, 512], F32, tag=f"pv{h01}", name=f"pv{h01}"
                            )
                            for h01 in range(2)
                        ]
                        prev_ex = None
                        for kt in range(NKT):
                            sss = [
                                psS.tile(
                                    [P, 512], F32, tag=f"ss{h01}", name=f"ss{h01}"
                                )
                                for h01 in range(2)
                            ]
                            for h01 in range(2):
                                pb_ = h01 * DK
                                nc.tensor.matmul(
                                    sss[h01],
                                    kt_t[pb_ : pb_ + DK, kt * P : (kt + 1) * P],
                                    QT[hp][
                                        pb_ : pb_ + DK,
                                        qt * 512 : (qt + 1) * 512,
                                    ],
                                    start=True,
                                    stop=True,
                                    tile_position=(pb_, 0),
                                )
                            # interleave next head pair's K projection into
                            # the PE slack of this one (ACT-bound loop)
                            if (
                                qt == 0
                                and kt in (2, 6, 10, 14)
                                and hp + 1 < NHP
                                and "noattn" not in ab
                            ):
                                if kt == 2:
                                    kproj_start(hp + 1)
                                kproj_chunk(hp + 1, kt // 4)
                            if "noexp" in ab:
                                exs = exf
                            else:
                                exs = []
                                for h01 in range(2):
                                    ex = pde.tile(
                                        [P, 512], BF16,
                                        tag=f"ex{h01}", name=f"ex{h01}",
                                    )
                                    nc.scalar.activation(ex, sss[h01], AF.Exp)
                                    exs.append(ex)
                            if prev_ex is not None:
                                pex, pkt = prev_ex
                                for h01 in range(2):
                                    nc.tensor.matmul(
                                        pv[h01],
                                        Vt[pkt][:, 2 * hp + h01, :],
                                        pex[h01],
                                        start=(pkt == 0),
                                        stop=False,
                                    )
                            prev_ex = (exs, kt)
                        pex, pkt = prev_ex
                        for h01 in range(2):
                            nc.tensor.matmul(
                                pv[h01],
                                Vt[pkt][:, 2 * hp + h01, :],
                                pex[h01],
                                start=False,
                                stop=True,
                            )
                        for h01 in range(2):
                            pb_ = h01 * DK
                            dst = xo_t[pb_ : pb_ + DK, qt * 512 : (qt + 1) * 512]
                            if "nonorm" in ab:
                                nc.vector.tensor_copy(dst, pv[h01][0:DK, :])
                                continue
                            rc = pdr.tile([1, 512], MM_DT, tag="rc", name="rc")
                            with nc.allow_low_precision(
                                reason="1/denom feeds f32r broadcast matmul"
                            ):
                                nc.vector.reciprocal(rc, pv[h01][DK : DK + 1, :])
                            rbp = psR.tile([DK, 512], F32, tag="rbp", name="rbp")
                            nc.tensor.matmul(rbp, ones_t, rc, start=True, stop=True)
                            nc.vector.tensor_copy(dst, pv[h01][0:DK, :])
                            nc.vector.tensor_mul(dst, dst, rbp)

                if "noattn" not in ab:
                    kproj_start(0)
                    for rt in range(NRT_K):
                        kproj_chunk(0, rt)
                    for hp in range(NHP):
                        attn(hp)

            pdx_cm.__exit__(None, None, None)

        # ---- Phase E: out = LN(x_o @ Wo^T + qres)  (bo, bv@Wo^T in qres)
        with (
            tc.tile_pool(name="pec", bufs=1) as pec,
            tc.tile_pool(name="peq", bufs=8) as peq,
            tc.tile_pool(name="pey", bufs=6) as pey,
            tc.tile_pool(name="pst", bufs=8) as pst,
            tc.tile_pool(name="psE", bufs=6, space="PSUM") as psE,
        ):
            g_b = pec.tile([P, D], F32)
            b_b = pec.tile([P, D], F32)
            eps_t = pec.tile([P, 1], F32)
            nc.sync.dma_start(g_b, gv[:].partition_broadcast(P))
            nc.sync.dma_start(b_b, bv2[:].partition_broadcast(P))
            nc.vector.memset(eps_t, 1e-5)
            xo = XO
            for rt in range(NRT_O):
                qr = peq.tile([P, D], F32)
                nc.gpsimd.dma_start(qr, qres[rt * P : (rt + 1) * P, :])
                y = pey.tile([P, D], F32)
                for o2 in range(2):
                    ps = psE.tile([P, 512], F32)
                    for hp in range(NOT):
                        nc.tensor.matmul(
                            ps,
                            xo[hp][:, rt * P : (rt + 1) * P],
                            wo[hp][:, o2 * 512 : (o2 + 1) * 512],
                            start=(hp == 0),
                            stop=(hp == NOT - 1),
                        )
                    nc.vector.tensor_add(
                        y[:, o2 * 512 : (o2 + 1) * 512],
                        ps,
                        qr[:, o2 * 512 : (o2 + 1) * 512],
                    )
                stats = pst.tile([P, 2, 6], F32)
                for sg in range(2):
                    nc.vector.bn_stats(
                        stats[:, sg, :], y[:, sg * 512 : (sg + 1) * 512]
                    )
                mv = pst.tile([P, 2], F32)
                nc.vector.bn_aggr(mv, stats)
                std = pst.tile([P, 1], F32)
                nc.scalar.activation(std, mv[:, 1:2], AF.Sqrt, bias=eps_t)
                rstd = pst.tile([P, 1], F32)
                nc.vector.reciprocal(rstd, std)
                nc.vector.tensor_scalar(
                    y,
                    y,
                    mv[:, 0:1],
                    rstd,
                    op0=ALU.subtract,
                    op1=ALU.mult,
                )
                eng = nc.vector if rt % 2 == 0 else nc.gpsimd
                eng.tensor_mul(y, y, g_b)
                eng.tensor_add(y, y, b_b)
                nc.sync.dma_start(out[rt * P : (rt + 1) * P, :], y)
        pwo_cm.__exit__(None, None, None)
        pxo_cm.__exit__(None, None, None)
    _split_sync_waits(nc)
    return nc


_NC = None


def _get_nc():
    global _NC
    if _NC is None:
        _NC = build_nc()
    return _NC


def prepare_in_maps(q, k, v, Wq, bq, Wk, bk, Wv, bv, Wo, bo, ln_g, ln_b):
    f = np.float32
    q = np.asarray(q, f)
    k = np.asarray(k, f)
    v = np.asarray(v, f)
    scale = 1.0 / np.sqrt(np.float32(DK))
    WqT = np.ascontiguousarray((np.asarray(Wq, f).T * scale).astype(ml_dtypes.bfloat16))
    WkT = np.ascontiguousarray(np.asarray(Wk, f).T.astype(ml_dtypes.bfloat16))
    WvT = np.ascontiguousarray(np.asarray(Wv, f).T.astype(ml_dtypes.bfloat16))
    WoT = np.ascontiguousarray(np.asarray(Wo, f).T.astype(ml_dtypes.bfloat16))
    bq_s = np.asarray(bq, f) * scale
    # bv flows through attention unchanged (probs sum to 1), so its effect on
    # the O projection is the constant vector bv @ Wo^T — fold into qres.
    res_const = np.asarray(bo, f) + np.asarray(bv, f) @ np.asarray(Wo, f).T
    common = {
        "WqT": WqT,
        "WkT": WkT,
        "WvT": WvT,
        "WoT": WoT,
        "bq": bq_s,
        "ln_g": np.asarray(ln_g, f),
        "ln_b": np.asarray(ln_b, f),
        "onesv": np.ones((P, NRT_V * H), ml_dtypes.bfloat16),
        "onesf": np.ones(DK, np.float32),
    }
    in_maps = []
    for c in range(8):
        b_, half = divmod(c, 2)
        qs = q[b_, half * M : (half + 1) * M, :]
        qres_c = qs + res_const[None, :]
        in_maps.append(
            dict(
                common,
                xqT=np.ascontiguousarray(qs.T.astype(ml_dtypes.bfloat16)),
                xkT=np.ascontiguousarray(k[b_].T.astype(ml_dtypes.bfloat16)),
                xvT=np.ascontiguousarray(v[b_].T.astype(ml_dtypes.bfloat16)),
                qres=np.ascontiguousarray(qres_c),
            )
        )
    return in_maps


def kernel(q, k, v, Wq, bq, Wk, bk, Wv, bv, Wo, bo, ln_g, ln_b):
    nc = _get_nc()
    in_maps = prepare_in_maps(q, k, v, Wq, bq, Wk, bk, Wv, bv, Wo, bo, ln_g, ln_b)
    res = run_bass_kernel_spmd(nc, in_maps, core_ids=list(range(8)))
    out = np.empty((B, S, D), np.float32)
    for c in range(8):
        b_, half = divmod(c, 2)
        out[b_, half * M : (half + 1) * M, :] = res.results[c]["out"]
    return out


# revision 14
# speedup vs baseline: 1.0188x; 1.0188x over previous
"""MultiHeadedAttention block (B=4, S=2048, D=1024, H=16) on 8 TRN2 cores.

Sharding: core c handles batch b=c//2 and query-row half c%2 (1024 rows).
Each core computes full K/V projections for its batch (2x redundant within a
batch pair), attention for all 16 heads over its 1024 query rows, then
O-projection + residual + LayerNorm. No collectives.

The real-HW bottleneck is the Activation engine's exp throughput
(~0.9 ns/elem, 33.5M softmax logits per core ~ 240 us). The attention inner
loop is software-pipelined in emission order so ACT never waits:
PE computes scores(k+1) while ACT does exp(k) while PE finishes PV(k-1),
with double-buffered score PSUM and a 4-deep exp-tile ring. K projection for
the next head pair is interleaved into the PE slack of the current one.

Device layouts (per core):
  Q^T  [o=1024, r=1024]  feature-major (partitions = features), per-ot tiles
  K^T  [o, k] projected per head pair inside the attention loop (no spill)
  V    [k=2048, o=1024]  row-major per-rt tiles, with a ones column per head
  scores computed transposed: S_t[k, q] = K_h^T Q_h  (softmax along k =
  partitions; exp without max-subtraction is safe: |logits| < ~3).
  P@V with the ones-augmented V gives the softmax denominator as row DK;
  normalization multiplies by a DMA-broadcast reciprocal.
Bias algebra: bk is dropped entirely (it shifts every logit of a softmax row
equally -> exactly cancels); bv and bo are folded into the residual tensor on
the host (exact by linearity of the O projection).
"""

import sys

if "/opt/trn_rl_repo" not in sys.path:
    sys.path.insert(0, "/opt/trn_rl_repo")

import ml_dtypes
import numpy as np

import concourse.bass as bass
import concourse.mybir as mybir
import concourse.tile as tile
from concourse.bass_utils import run_bass_kernel_spmd

B, S, D, H, DK = 4, 2048, 1024, 16, 64
P = 128
M = S // 2          # query rows per core
NDT = D // P        # 8 contraction chunks
NOT = D // P        # 8 output-feature chunks (= head pairs)
NHP = H // 2        # 8 head pairs
NKT = S // P        # 16 key chunks
NQT = M // 512      # 2 query 512-chunks
NRT_K = S // 512    # 4 key-row 512-chunks
NRT_V = S // P      # 16 V row chunks
NRT_O = M // P      # 8 output row chunks
F32 = mybir.dt.float32
MM_DT = mybir.dt.float32r
AF = mybir.ActivationFunctionType
ALU = mybir.AluOpType
BF16 = mybir.dt.bfloat16


def _split_sync_waits(nc, max_waits=1):
    """Split instructions carrying more than max_waits sem waits.

    The container's walrus rejects instructions with multiple sync wait
    commands, so excess waits move onto NoOp instructions inserted just
    before, on the same engine.
    """
    idx = 0
    for f in nc.m.functions:
        for blk in f.blocks:
            newl = []
            for inst in blk.instructions:
                si = inst.sync_info
                waits = list(si.on_wait) if si is not None and si.on_wait else []
                if len(waits) > max_waits:
                    extra = waits[max_waits:]
                    si.on_wait = waits[:max_waits]
                    for j in range(0, len(extra), max_waits):
                        nop = mybir.InstNoOp(name=f"I-wsplit-{idx}", ins=[], outs=[])
                        idx += 1
                        nop.engine = inst.engine
                        nop.sync_info = mybir.SyncInfo(
                            on_wait=extra[j : j + max_waits], on_update=[]
                        )
                        newl.append(nop)
                newl.append(inst)
            blk.instructions = newl


def build_nc(loops=0, unroll=1, ab=()):
    nc = bass.Bass()
    xqT = nc.dram_tensor("xqT", [D, M], BF16, kind="ExternalInput")
    xkT = nc.dram_tensor("xkT", [D, S], BF16, kind="ExternalInput")
    xvT = nc.dram_tensor("xvT", [D, S], BF16, kind="ExternalInput")
    qres = nc.dram_tensor("qres", [M, D], F32, kind="ExternalInput")
    WqT = nc.dram_tensor("WqT", [D, D], BF16, kind="ExternalInput")
    WkT = nc.dram_tensor("WkT", [D, D], BF16, kind="ExternalInput")
    WvT = nc.dram_tensor("WvT", [D, D], BF16, kind="ExternalInput")
    WoT = nc.dram_tensor("WoT", [D, D], BF16, kind="ExternalInput")
    bqv = nc.dram_tensor("bq", [D], F32, kind="ExternalInput")
    gv = nc.dram_tensor("ln_g", [D], F32, kind="ExternalInput")
    bv2 = nc.dram_tensor("ln_b", [D], F32, kind="ExternalInput")
    onesv = nc.dram_tensor("onesv", [P, NRT_V * H], BF16, kind="ExternalInput")
    onesf = nc.dram_tensor("onesf", [DK], F32, kind="ExternalInput")
    out = nc.dram_tensor("out", [M, D], F32, kind="ExternalOutput")

    WqT_r = WqT[:, :].rearrange("(a p) o -> p a o", p=P)
    WkT_r = WkT[:, :].rearrange("(a p) o -> p a o", p=P)
    WvT_r = WvT[:, :].rearrange("(a p) o -> p a o", p=P)
    WoT_r = WoT[:, :].rearrange("(a p) o -> p a o", p=P)
    xqT_r = xqT[:, :].rearrange("(a p) r -> p a r", p=P)
    xkT_r = xkT[:, :].rearrange("(a p) r -> p a r", p=P)
    xvT_r = xvT[:, :].rearrange("(a p) r -> p a r", p=P)

    with tile.TileContext(nc) as tc:
      for _rep in range(max(1, unroll)):
        pxo_cm = tc.tile_pool(name="pxo", bufs=1)
        pxo = pxo_cm.__enter__()
        with (
            tc.tile_pool(name="pqv", bufs=1) as pqv,
        ):
            XO = [
                pxo.tile([P, M], BF16, tag=f"XO{i}", name=f"XO{i}")
                for i in range(NHP)
            ]
            if "noattn" in ab:
                for t in XO:
                    nc.vector.memset(t, 0.001)
            exf = None
            if "noexp" in ab:
                exf = pxo.tile([P, 2, 512], BF16, tag="exf", name="exf")
                nc.vector.memset(exf, 0.001)

            QT = []
            for ot in range(NOT):
                t = pqv.tile([P, M], BF16, tag=f"QT{ot}", name=f"QT{ot}")
                QT.append(t)
            bq_p = pqv.tile([P, NOT], F32)
            nc.gpsimd.dma_start(bq_p, bqv[:].rearrange("(a p) -> p a", p=P))
            ones_t = pqv.tile([1, DK], MM_DT)
            nc.gpsimd.dma_start(
                ones_t, onesf[:].partition_broadcast(1).bitcast(MM_DT)
            )
            Vt = []
            for rt in range(NRT_V):
                t = pqv.tile([P, H, DK + 1], BF16, tag=f"Vt{rt}", name=f"Vt{rt}")
                nc.gpsimd.dma_start(
                    t[:, :, DK : DK + 1],
                    onesv[:, rt * H : (rt + 1) * H],
                )
                Vt.append(t)
            # wv loads early so phase B starts without a DMA stall
            pwv_cm = tc.tile_pool(name="pwv", bufs=NDT, side="right")
            pwv = pwv_cm.__enter__()
            wv = []
            for dt in range(NDT):
                w_t = pwv.tile([P, D], BF16, tag="wv", name=f"wv{dt}")
                nc.gpsimd.dma_start(w_t, WvT_r[:, dt, :])
                wv.append(w_t)

            pbx_cm = tc.tile_pool(name="pbx", bufs=3, side="right")
            pbx = pbx_cm.__enter__()
            psAB_cm = tc.tile_pool(name="psAB", bufs=8, space="PSUM")
            psAB = psAB_cm.__enter__()

            # ---- Phase A: Q^T = (Wq/8) @ x_q^T + bq/8, layout [o, r]
            with (
                tc.tile_pool(name="pa", bufs=NDT) as pa,
            ):
                wq = []
                xq = []
                xv_pre = {}
                for dt in range(NDT):
                    w_t = pa.tile([P, D], BF16, tag="wq", name=f"wq{dt}")
                    nc.sync.dma_start(w_t, WqT_r[:, dt, :])
                    wq.append(w_t)
                    x_t = pa.tile([P, M], BF16, tag="xq", name=f"xq{dt}")
                    nc.sync.dma_start(x_t, xqT_r[:, dt, :])
                    xq.append(x_t)
                    if dt in (2, 4, 6):
                        rt = dt // 2 - 1
                        xv_t = pbx.tile(
                            [P, NDT, P], BF16, tag="xv", name="xv"
                        )
                        nc.sync.dma_start(
                            xv_t, xvT_r[:, :, rt * P : (rt + 1) * P]
                        )
                        xv_pre[rt] = xv_t
                for ot in range(NOT):
                    for qt in range(NQT):
                        ps = psAB.tile([P, 512], F32, tag='ps', name='ps')
                        for dt in range(NDT):
                            nc.tensor.matmul(
                                ps,
                                wq[dt][:, ot * P : (ot + 1) * P],
                                xq[dt][:, qt * 512 : (qt + 1) * 512],
                                start=(dt == 0),
                                stop=(dt == NDT - 1),
                            )
                        nc.vector.tensor_scalar_add(
                            QT[ot][:, qt * 512 : (qt + 1) * 512],
                            ps,
                            bq_p[:, ot : ot + 1],
                        )

            # xk loads during phase B so phase D starts without a DMA stall
            pdx_cm = tc.tile_pool(name="pdx", bufs=NDT)
            pdx = pdx_cm.__enter__()
            xk = []
            for dt in range(NDT):
                x_t = pdx.tile([P, S], BF16, tag="xk", name=f"xk{dt}")
                nc.gpsimd.dma_start(x_t, xkT_r[:, dt, :])
                xk.append(x_t)

            # ---- Phase B: V = x_v @ Wv^T (bias folded into qres), [r, o]
            for rt in range(NRT_V):
                if rt in xv_pre:
                    xv = xv_pre.pop(rt)
                else:
                    xv = pbx.tile(
                        [P, NDT, P], BF16, tag="xv", name="xv"
                    )
                    veng = nc.sync if rt < 6 else nc.gpsimd
                    veng.dma_start(xv, xvT_r[:, :, rt * P : (rt + 1) * P])
                for o2 in range(2):
                    ps = psAB.tile([P, 512], F32, tag='ps', name='ps')
                    for dt in range(NDT):
                        nc.tensor.matmul(
                            ps,
                            xv[:, dt, :],
                            wv[dt][:, o2 * 512 : (o2 + 1) * 512],
                            start=(dt == 0),
                            stop=(dt == NDT - 1),
                        )
                    nc.vector.tensor_copy(
                        Vt[rt][:, o2 * 8 : (o2 + 1) * 8, 0:DK],
                        ps[:, :].rearrange("p (h e) -> p h e", e=DK),
                    )

            pbx_cm.__exit__(None, None, None)
            pwv_cm.__exit__(None, None, None)
            psAB_cm.__exit__(None, None, None)

            # wo prefetch during D so phase E starts without a DMA stall
            pwo_cm = tc.tile_pool(name="pwo", bufs=NDT, side="right")
            pwo = pwo_cm.__enter__()
            wo = []
            for dt in range(NDT):
                w_t = pwo.tile([P, D], BF16, tag="wo", name=f"wo{dt}")
                nc.gpsimd.dma_start(w_t, WoT_r[:, dt, :])
                wo.append(w_t)

            # ---- Phase D: K^T projection fused with attention, per head pair.
            # Software-pipelined per (qt, h01) section of 8 KG=2 groups:
            # PE scores(g+1) || ACT exp(g) || PE pv(g-1). PV PSUM and score
            # PSUM are double-buffered so section boundaries don't stall, and
            # the normalize chain is deferred into the next section's slack.
            KG = 2
            NG = NKT // KG
            with (
                tc.tile_pool(name="pdw", bufs=2) as pdw,
                tc.tile_pool(name="pdkt", bufs=2) as pdkt,
                tc.tile_pool(name="pde", bufs=4) as pde,
                tc.tile_pool(name="pdr", bufs=2) as pdr,
                tc.tile_pool(name="psS", bufs=2, space="PSUM") as psS,
                tc.tile_pool(name="psK", bufs=1, space="PSUM") as psK,
                tc.tile_pool(name="psR", bufs=1, space="PSUM") as psR,
                tc.tile_pool(name="psPV", bufs=2, space="PSUM") as psPV,
            ):
                kts = {}
                wks = {}
                pending = []

                def flush_pending():
                    while pending:
                        pending.pop(0)()

                def kproj_start(hp):
                    wk = pdw.tile([P, NDT, P], BF16, tag="wk", name="wk")
                    nc.sync.dma_start(wk, WkT_r[:, :, hp * P : (hp + 1) * P])
                    wks[hp] = wk
                    kts[hp] = pdkt.tile([P, S], BF16, tag="kt", name="kt")

                def kproj_chunk(hp, rt):
                    wk = wks[hp]
                    kt_t = kts[hp]
                    ps = psK.tile([P, 512], F32, tag="kps", name="kps")
                    for dt in range(NDT):
                        nc.tensor.matmul(
                            ps,
                            wk[:, dt, :],
                            xk[dt][:, rt * 512 : (rt + 1) * 512],
                            start=(dt == 0),
                            stop=(dt == NDT - 1),
                        )
                    nc.vector.tensor_copy(
                        kt_t[:, rt * 512 : (rt + 1) * 512], ps
                    )

                def section(hp, qt, h01, kp_slots):
                    kt_t = kts[hp]
                    head = 2 * hp + h01
                    pb_ = h01 * DK
                    qsl = slice(qt * 512, (qt + 1) * 512)
                    pv = psPV.tile([DK + 1, 512], F32, tag="pv", name="pv")
                    prev = None
                    for g in range(NG):
                        ss = psS.tile([P, KG, 512], F32, tag="ss", name="ss")
                        for j in range(KG):
                            kt = g * KG + j
                            nc.tensor.matmul(
                                ss[:, j, :],
                                kt_t[pb_ : pb_ + DK, kt * P : (kt + 1) * P],
                                QT[hp][pb_ : pb_ + DK, qsl],
                                start=True,
                                stop=True,
                                tile_position=(pb_, 0),
                            )
                        if g == 1:
                            flush_pending()
                        if g in kp_slots:
                            if kp_slots[g] == "start":
                                kproj_start(hp + 1)
                            else:
                                kproj_chunk(hp + 1, kp_slots[g])
                        if "noexp" in ab:
                            ex = exf
                        else:
                            ex = pde.tile(
                                [P, KG, 512], BF16, tag="ex", name="ex"
                            )
                            nc.scalar.activation(ex, ss, AF.Exp)
                        if prev is not None:
                            pex, pg = prev
                            for j in range(KG):
                                kt = pg * KG + j
                                nc.tensor.matmul(
                                    pv,
                                    Vt[kt][:, head, :],
                                    pex[:, j, :],
                                    start=(kt == 0),
                                    stop=False,
                                )
                        prev = (ex, g)
                    pex, pg = prev
                    for j in range(KG):
                        kt = pg * KG + j
                        nc.tensor.matmul(
                            pv,
                            Vt[kt][:, head, :],
                            pex[:, j, :],
                            start=False,
                            stop=(kt == NKT - 1),
                        )
                    dst = XO[hp][pb_ : pb_ + DK, qsl]
                    if "nonorm" in ab:
                        nc.vector.tensor_copy(dst, pv[0:DK, :])
                        return
                    rc = pdr.tile([1, 512], MM_DT, tag="rc", name="rc")
                    with nc.allow_low_precision(
                        reason="1/denom feeds f32r broadcast matmul"
                    ):
                        nc.vector.reciprocal(rc, pv[DK : DK + 1, :])

                    def norm(pv=pv, rc=rc, dst=dst):
                        rbp = psR.tile([DK, 512], F32, tag="rbp", name="rbp")
                        nc.tensor.matmul(rbp, ones_t, rc, start=True, stop=True)
                        nc.vector.tensor_copy(dst, pv[0:DK, :])
                        nc.vector.tensor_mul(dst, dst, rbp)

                    pending.append(norm)

                if "noattn" not in ab:
                    kproj_start(0)
                    for rt in range(NRT_K):
                        kproj_chunk(0, rt)
                    for hp in range(NHP):
                        kp = {}
                        if hp + 1 < NHP:
                            kp = {
                                (0, 0): {1: "start", 3: 0, 6: 1},
                                (0, 1): {3: 2, 6: 3},
                            }
                        for qt in range(NQT):
                            for h01 in range(2):
                                section(hp, qt, h01, kp.get((qt, h01), {}))
                        kts.pop(hp, None)
                        wks.pop(hp, None)
                    flush_pending()

            pdx_cm.__exit__(None, None, None)

        # ---- Phase E: out = LN(x_o @ Wo^T + qres)  (bo, bv@Wo^T in qres)
        with (
            tc.tile_pool(name="pec", bufs=1) as pec,
            tc.tile_pool(name="peq", bufs=8) as peq,
            tc.tile_pool(name="pey", bufs=6) as pey,
            tc.tile_pool(name="pst", bufs=8) as pst,
            tc.tile_pool(name="psE", bufs=6, space="PSUM") as psE,
        ):
            g_b = pec.tile([P, D], F32)
            b_b = pec.tile([P, D], F32)
            eps_t = pec.tile([P, 1], F32)
            nc.sync.dma_start(g_b, gv[:].partition_broadcast(P))
            nc.sync.dma_start(b_b, bv2[:].partition_broadcast(P))
            nc.vector.memset(eps_t, 1e-5)
            xo = XO
            for rt in range(NRT_O):
                qr = peq.tile([P, D], F32)
                nc.gpsimd.dma_start(qr, qres[rt * P : (rt + 1) * P, :])
                y = pey.tile([P, D], F32)
                for o2 in range(2):
                    ps = psE.tile([P, 512], F32)
                    for hp in range(NOT):
                        nc.tensor.matmul(
                            ps,
                            xo[hp][:, rt * P : (rt + 1) * P],
                            wo[hp][:, o2 * 512 : (o2 + 1) * 512],
                            start=(hp == 0),
                            stop=(hp == NOT - 1),
                        )
                    nc.vector.tensor_add(
                        y[:, o2 * 512 : (o2 + 1) * 512],
                        ps,
                        qr[:, o2 * 512 : (o2 + 1) * 512],
                    )
                stats = pst.tile([P, 2, 6], F32)
                for sg in range(2):
                    nc.vector.bn_stats(
                        stats[:, sg, :], y[:, sg * 512 : (sg + 1) * 512]
                    )
                mv = pst.tile([P, 2], F32)
                nc.vector.bn_aggr(mv, stats)
                std = pst.tile([P, 1], F32)
                nc.scalar.activation(std, mv[:, 1:2], AF.Sqrt, bias=eps_t)
                rstd = pst.tile([P, 1], F32)
                nc.vector.reciprocal(rstd, std)
                nc.vector.tensor_scalar(
                    y,
                    y,
                    mv[:, 0:1],
                    rstd,
                    op0=ALU.subtract,
                    op1=ALU.mult,
                )
                eng = nc.vector if rt % 2 == 0 else nc.gpsimd
                eng.tensor_mul(y, y, g_b)
                eng.tensor_add(y, y, b_b)
                nc.sync.dma_start(out[rt * P : (rt + 1) * P, :], y)
        pwo_cm.__exit__(None, None, None)
        pxo_cm.__exit__(None, None, None)
    _split_sync_waits(nc)
    return nc


_NC = None


def _get_nc():
    global _NC
    if _NC is None:
        _NC = build_nc()
    return _NC


def prepare_in_maps(q, k, v, Wq, bq, Wk, bk, Wv, bv, Wo, bo, ln_g, ln_b):
    f = np.float32
    q = np.asarray(q, f)
    k = np.asarray(k, f)
    v = np.asarray(v, f)
    scale = 1.0 / np.sqrt(np.float32(DK))
    WqT = np.ascontiguousarray((np.asarray(Wq, f).T * scale).astype(ml_dtypes.bfloat16))
    WkT = np.ascontiguousarray(np.asarray(Wk, f).T.astype(ml_dtypes.bfloat16))
    WvT = np.ascontiguousarray(np.asarray(Wv, f).T.astype(ml_dtypes.bfloat16))
    WoT = np.ascontiguousarray(np.asarray(Wo, f).T.astype(ml_dtypes.bfloat16))
    bq_s = np.asarray(bq, f) * scale
    # bv flows through attention unchanged (probs sum to 1), so its effect on
    # the O projection is the constant vector bv @ Wo^T — fold into qres.
    res_const = np.asarray(bo, f) + np.asarray(bv, f) @ np.asarray(Wo, f).T
    common = {
        "WqT": WqT,
        "WkT": WkT,
        "WvT": WvT,
        "WoT": WoT,
        "bq": bq_s,
        "ln_g": np.asarray(ln_g, f),
        "ln_b": np.asarray(ln_b, f),
        "onesv": np.ones((P, NRT_V * H), ml_dtypes.bfloat16),
        "onesf": np.ones(DK, np.float32),
    }
    in_maps = []
    for c in range(8):
        b_, half = divmod(c, 2)
        qs = q[b_, half * M : (half + 1) * M, :]
        qres_c = qs + res_const[None, :]
        in_maps.append(
            dict(
                common,
                xqT=np.ascontiguousarray(qs.T.astype(ml_dtypes.bfloat16)),
                xkT=np.ascontiguousarray(k[b_].T.astype(ml_dtypes.bfloat16)),
                xvT=np.ascontiguousarray(v[b_].T.astype(ml_dtypes.bfloat16)),
                qres=np.ascontiguousarray(qres_c),
            )
        )
    return in_maps


def kernel(q, k, v, Wq, bq, Wk, bk, Wv, bv, Wo, bo, ln_g, ln_b):
    nc = _get_nc()
    in_maps = prepare_in_maps(q, k, v, Wq, bq, Wk, bk, Wv, bv, Wo, bo, ln_g, ln_b)
    res = run_bass_kernel_spmd(nc, in_maps, core_ids=list(range(8)))
    out = np.empty((B, S, D), np.float32)
    for c in range(8):
        b_, half = divmod(c, 2)
        out[b_, half * M : (half + 1) * M, :] = res.results[c]["out"]
    return out


# revision 19
# speedup vs baseline: 1.2332x; 1.2104x over previous
"""MultiHeadedAttention block (B=4, S=2048, D=1024, H=16) on 8 TRN2 cores.

Sharding: core c handles batch b=c//2 and query-row half c%2 (1024 rows).
Each core computes full K/V projections for its batch (2x redundant within a
batch pair), attention for all 16 heads over its 1024 query rows, then
O-projection + residual + LayerNorm. No collectives.

The real-HW bottleneck is the Activation engine's exp throughput
(~0.9 ns/elem, 33.5M softmax logits per core ~ 240 us). The attention inner
loop is software-pipelined in emission order so ACT never waits:
PE computes scores(k+1) while ACT does exp(k) while PE finishes PV(k-1),
with double-buffered score PSUM and a 4-deep exp-tile ring. K projection for
the next head pair is interleaved into the PE slack of the current one.

Device layouts (per core):
  Q^T  [o=1024, r=1024]  feature-major (partitions = features), per-ot tiles
  K^T  [o, k] projected per head pair inside the attention loop (no spill)
  V    [k=2048, o=1024]  row-major per-rt tiles, with a ones column per head
  scores computed transposed: S_t[k, q] = K_h^T Q_h  (softmax along k =
  partitions; exp without max-subtraction is safe: |logits| < ~3).
  P@V with the ones-augmented V gives the softmax denominator as row DK;
  normalization multiplies by a DMA-broadcast reciprocal.
Bias algebra: bk is dropped entirely (it shifts every logit of a softmax row
equally -> exactly cancels); bv and bo are folded into the residual tensor on
the host (exact by linearity of the O projection).
"""

import sys

if "/opt/trn_rl_repo" not in sys.path:
    sys.path.insert(0, "/opt/trn_rl_repo")

import ml_dtypes
import numpy as np

import concourse.bass as bass
import concourse.mybir as mybir
import concourse.tile as tile
from concourse.bass_utils import run_bass_kernel_spmd

B, S, D, H, DK = 4, 2048, 1024, 16, 64
P = 128
M = S // 2          # query rows per core
NDT = D // P        # 8 contraction chunks
NOT = D // P        # 8 output-feature chunks (= head pairs)
NHP = H // 2        # 8 head pairs
NKT = S // P        # 16 key chunks
NQT = M // 512      # 2 query 512-chunks
NRT_K = S // 512    # 4 key-row 512-chunks
NRT_V = S // P      # 16 V row chunks
NRT_O = M // P      # 8 output row chunks
F32 = mybir.dt.float32
MM_DT = mybir.dt.float32r
AF = mybir.ActivationFunctionType
ALU = mybir.AluOpType
BF16 = mybir.dt.bfloat16
F8 = mybir.dt.float8e4


def _split_sync_waits(nc, max_waits=1):
    """Split instructions carrying more than max_waits sem waits.

    The container's walrus rejects instructions with multiple sync wait
    commands, so excess waits move onto NoOp instructions inserted just
    before, on the same engine.
    """
    idx = 0
    for f in nc.m.functions:
        for blk in f.blocks:
            newl = []
            for inst in blk.instructions:
                si = inst.sync_info
                waits = list(si.on_wait) if si is not None and si.on_wait else []
                if len(waits) > max_waits:
                    extra = waits[max_waits:]
                    si.on_wait = waits[:max_waits]
                    for j in range(0, len(extra), max_waits):
                        nop = mybir.InstNoOp(name=f"I-wsplit-{idx}", ins=[], outs=[])
                        idx += 1
                        nop.engine = inst.engine
                        nop.sync_info = mybir.SyncInfo(
                            on_wait=extra[j : j + max_waits], on_update=[]
                        )
                        newl.append(nop)
                newl.append(inst)
            blk.instructions = newl


def build_nc(loops=0, unroll=1, ab=()):
    nc = bass.Bass()
    NDR = D // 256
    xqT = nc.dram_tensor("xqT", [NDR, P, 2, M], F8, kind="ExternalInput")
    xkT = nc.dram_tensor("xkT", [NDR, P, 2, S], F8, kind="ExternalInput")
    xvT = nc.dram_tensor("xvT", [NDR, P, 2, S], F8, kind="ExternalInput")
    qres = nc.dram_tensor("qres", [M, D], F32, kind="ExternalInput")
    WqT = nc.dram_tensor("WqT", [NDR, P, 2, D], F8, kind="ExternalInput")
    WkT = nc.dram_tensor("WkT", [NDR, P, 2, D], F8, kind="ExternalInput")
    WvT = nc.dram_tensor("WvT", [NDR, P, 2, D], F8, kind="ExternalInput")
    WoT = nc.dram_tensor("WoT", [D, D], BF16, kind="ExternalInput")
    bqv = nc.dram_tensor("bq", [D], F32, kind="ExternalInput")
    gv = nc.dram_tensor("ln_g", [D], F32, kind="ExternalInput")
    bv2 = nc.dram_tensor("ln_b", [D], F32, kind="ExternalInput")
    ones8 = nc.dram_tensor("onesv", [P, NRT_V * H], F8, kind="ExternalInput")
    onesf = nc.dram_tensor("onesf", [DK], F32, kind="ExternalInput")
    out = nc.dram_tensor("out", [M, D], F32, kind="ExternalOutput")

    WoT_r = WoT[:, :].rearrange("(a p) o -> p a o", p=P)

    with tile.TileContext(nc) as tc:
      for _rep in range(max(1, unroll)):
        pxo_cm = tc.tile_pool(name="pxo", bufs=1)
        pxo = pxo_cm.__enter__()
        with (
            tc.tile_pool(name="pqv", bufs=1) as pqv,
        ):
            XO = [
                pxo.tile([P, M], BF16, tag=f"XO{i}", name=f"XO{i}")
                for i in range(NHP)
            ]
            if "noattn" in ab:
                for t in XO:
                    nc.vector.memset(t, 0.001)
            exf = None
            if "noexp" in ab:
                exf = pxo.tile([P, 2, 512], F8, tag="exf", name="exf")
                nc.vector.memset(exf, 0.001)

            QT = []
            for ot in range(NOT):
                t = pqv.tile([P, M], BF16, tag=f"QT{ot}", name=f"QT{ot}")
                QT.append(t)
            bq_p = pqv.tile([P, NOT], F32)
            nc.gpsimd.dma_start(bq_p, bqv[:].rearrange("(a p) -> p a", p=P))
            ones_t = pqv.tile([1, DK], MM_DT)
            nc.gpsimd.dma_start(
                ones_t, onesf[:].partition_broadcast(1).bitcast(MM_DT)
            )
            Vp = []
            for rtp in range(NRT_V // 2):
                t = pqv.tile(
                    [P, 2, H, DK + 1], F8, tag=f"Vp{rtp}", name=f"Vp{rtp}"
                )
                for j in range(2):
                    nc.gpsimd.dma_start(
                        t[:, j, :, DK : DK + 1],
                        ones8[:, (2 * rtp + j) * H : (2 * rtp + j + 1) * H],
                    )
                Vp.append(t)
            # wv loads early so phase B starts without a DMA stall
            pwv_cm = tc.tile_pool(name="pwv", bufs=1, side="right")
            pwv = pwv_cm.__enter__()
            wv = []
            for g in range(NDR):
                w_t = pwv.tile([P, 2, D], F8, tag=f"wv{g}", name=f"wv{g}")
                nc.gpsimd.dma_start(w_t, WvT[g, :, :, :])
                wv.append(w_t)

            pbx_cm = tc.tile_pool(name="pbx", bufs=1, side="right")
            pbx = pbx_cm.__enter__()
            psAB_cm = tc.tile_pool(name="psAB", bufs=8, space="PSUM")
            psAB = psAB_cm.__enter__()

            # ---- Phase A: Q^T = (Wq/8) @ x_q^T + bq/8, layout [o, r]
            with (
                tc.tile_pool(name="pa", bufs=1) as pa,
            ):
                wq = []
                xq = []
                for g in range(NDR):
                    w_t = pa.tile([P, 2, D], F8, tag=f"wq{g}", name=f"wq{g}")
                    nc.sync.dma_start(w_t, WqT[g, :, :, :])
                    wq.append(w_t)
                    x_t = pa.tile([P, 2, M], F8, tag=f"xq{g}", name=f"xq{g}")
                    nc.sync.dma_start(x_t, xqT[g, :, :, :])
                    xq.append(x_t)
                xv = []
                for g in range(NDR):
                    x_t = pbx.tile([P, 2, S], F8, tag=f"xv{g}", name=f"xv{g}")
                    nc.sync.dma_start(x_t, xvT[g, :, :, :])
                    xv.append(x_t)
                for ot in range(NOT):
                    for qt in range(NQT):
                        ps = psAB.tile([P, 512], F32, tag='ps', name='ps')
                        for g in range(NDR):
                            nc.tensor.matmul(
                                ps,
                                wq[g][:, :, ot * P : (ot + 1) * P],
                                xq[g][:, :, qt * 512 : (qt + 1) * 512],
                                start=(g == 0),
                                stop=(g == NDR - 1),
                                perf_mode=mybir.MatmulPerfMode.DoubleRow,
                            )
                        nc.vector.tensor_scalar_add(
                            QT[ot][:, qt * 512 : (qt + 1) * 512],
                            ps,
                            bq_p[:, ot : ot + 1],
                        )

            # xk/wk load during phase B so phase D starts without a DMA stall
            pdx_cm = tc.tile_pool(name="pdx", bufs=1)
            pdx = pdx_cm.__enter__()
            xk = []
            wk = []
            for g in range(NDR):
                x_t = pdx.tile([P, 2, S], F8, tag=f"xk{g}", name=f"xk{g}")
                nc.gpsimd.dma_start(x_t, xkT[g, :, :, :])
                xk.append(x_t)
            for g in range(NDR):
                w_t = pdx.tile([P, 2, D], F8, tag=f"wk{g}", name=f"wk{g}")
                nc.gpsimd.dma_start(w_t, WkT[g, :, :, :])
                wk.append(w_t)

            # ---- Phase B: V = x_v @ Wv^T (bias folded into qres), [r, o] fp8 pairs
            for rt in range(NRT_V):
                for o2 in range(2):
                    ps = psAB.tile([P, 512], F32, tag='ps', name='ps')
                    for g in range(NDR):
                        nc.tensor.matmul(
                            ps,
                            xv[g][:, :, rt * P : (rt + 1) * P],
                            wv[g][:, :, o2 * 512 : (o2 + 1) * 512],
                            start=(g == 0),
                            stop=(g == NDR - 1),
                            perf_mode=mybir.MatmulPerfMode.DoubleRow,
                        )
                    nc.vector.tensor_copy(
                        Vp[rt // 2][:, rt % 2, o2 * 8 : (o2 + 1) * 8, 0:DK],
                        ps[:, :].rearrange("p (h e) -> p h e", e=DK),
                    )

            pbx_cm.__exit__(None, None, None)
            pwv_cm.__exit__(None, None, None)
            psAB_cm.__exit__(None, None, None)

            # wo prefetch during D so phase E starts without a DMA stall
            pwo_cm = tc.tile_pool(name="pwo", bufs=NDT, side="right")
            pwo = pwo_cm.__enter__()
            wo = []
            for dt in range(NDT):
                w_t = pwo.tile([P, D], BF16, tag="wo", name=f"wo{dt}")
                nc.gpsimd.dma_start(w_t, WoT_r[:, dt, :])
                wo.append(w_t)

            # ---- Phase D: K^T projection fused with attention, per head pair.
            # Software-pipelined per (qt, h01) section of 8 KG=2 groups:
            # PE scores(g+1) || ACT exp(g) || PE pv(g-1). PV PSUM and score
            # PSUM are double-buffered so section boundaries don't stall, and
            # the normalize chain is deferred into the next section's slack.
            KG = 2
            NG = NKT // KG
            with (
                tc.tile_pool(name="pdkt", bufs=2) as pdkt,
                tc.tile_pool(name="pde", bufs=4) as pde,
                tc.tile_pool(name="pdr", bufs=2) as pdr,
                tc.tile_pool(name="psS", bufs=2, space="PSUM") as psS,
                tc.tile_pool(name="psK", bufs=1, space="PSUM") as psK,
                tc.tile_pool(name="psR", bufs=1, space="PSUM") as psR,
                tc.tile_pool(name="psPV", bufs=2, space="PSUM") as psPV,
            ):
                kts = {}
                pending = []

                def flush_pending():
                    while pending:
                        pending.pop(0)()

                def kproj_start(hp):
                    kts[hp] = pdkt.tile([P, S], BF16, tag="kt", name="kt")

                def kproj_chunk(hp, rt):
                    kt_t = kts[hp]
                    ps = psK.tile([P, 512], F32, tag="kps", name="kps")
                    for g in range(NDR):
                        nc.tensor.matmul(
                            ps,
                            wk[g][:, :, hp * P : (hp + 1) * P],
                            xk[g][:, :, rt * 512 : (rt + 1) * 512],
                            start=(g == 0),
                            stop=(g == NDR - 1),
                            perf_mode=mybir.MatmulPerfMode.DoubleRow,
                        )
                    nc.vector.tensor_copy(
                        kt_t[:, rt * 512 : (rt + 1) * 512], ps
                    )

                def section(hp, qt, h01, kp_slots):
                    kt_t = kts[hp]
                    head = 2 * hp + h01
                    pb_ = h01 * DK
                    qsl = slice(qt * 512, (qt + 1) * 512)
                    pv = psPV.tile([DK + 1, 512], F32, tag="pv", name="pv")
                    prev = None
                    for g in range(NG):
                        ss = psS.tile([P, KG, 512], F32, tag="ss", name="ss")
                        for j in range(KG):
                            kt = g * KG + j
                            nc.tensor.matmul(
                                ss[:, j, :],
                                kt_t[pb_ : pb_ + DK, kt * P : (kt + 1) * P],
                                QT[hp][pb_ : pb_ + DK, qsl],
                                start=True,
                                stop=True,
                                tile_position=(pb_, 0),
                            )
                        if g == 1:
                            flush_pending()
                        if g in kp_slots:
                            if kp_slots[g] == "start":
                                kproj_start(hp + 1)
                            else:
                                kproj_chunk(hp + 1, kp_slots[g])
                        if "noexp" in ab:
                            ex = exf
                        else:
                            ex = pde.tile(
                                [P, KG, 512], F8, tag="ex", name="ex"
                            )
                            nc.scalar.activation(ex, ss, AF.Exp)
                        if prev is not None:
                            pex, pg = prev
                            nc.tensor.matmul(
                                pv,
                                Vp[pg][:, :, head, :],
                                pex,
                                start=(pg == 0),
                                stop=False,
                                perf_mode=mybir.MatmulPerfMode.DoubleRow,
                            )
                        prev = (ex, g)
                    pex, pg = prev
                    nc.tensor.matmul(
                        pv,
                        Vp[pg][:, :, head, :],
                        pex,
                        start=False,
                        stop=(pg == NG - 1),
                        perf_mode=mybir.MatmulPerfMode.DoubleRow,
                    )
                    dst = XO[hp][pb_ : pb_ + DK, qsl]
                    if "nonorm" in ab:
                        nc.vector.tensor_copy(dst, pv[0:DK, :])
                        return
                    rc = pdr.tile([1, 512], MM_DT, tag="rc", name="rc")
                    with nc.allow_low_precision(
                        reason="1/denom feeds f32r broadcast matmul"
                    ):
                        nc.vector.reciprocal(rc, pv[DK : DK + 1, :])

                    def norm(pv=pv, rc=rc, dst=dst):
                        rbp = psR.tile([DK, 512], F32, tag="rbp", name="rbp")
                        nc.tensor.matmul(rbp, ones_t, rc, start=True, stop=True)
                        nc.vector.tensor_copy(dst, pv[0:DK, :])
                        nc.vector.tensor_mul(dst, dst, rbp)

                    pending.append(norm)

                if "noattn" not in ab:
                    kproj_start(0)
                    for rt in range(NRT_K):
                        kproj_chunk(0, rt)
                    for hp in range(NHP):
                        kp = {}
                        if hp + 1 < NHP:
                            kp = {
                                (0, 0): {1: "start", 3: 0, 6: 1},
                                (0, 1): {3: 2, 6: 3},
                            }
                        for qt in range(NQT):
                            for h01 in range(2):
                                section(hp, qt, h01, kp.get((qt, h01), {}))
                        kts.pop(hp, None)
                    flush_pending()

            pdx_cm.__exit__(None, None, None)

        # ---- Phase E: out = LN(x_o @ Wo^T + qres)  (bo, bv@Wo^T in qres)
        with (
            tc.tile_pool(name="pec", bufs=1) as pec,
            tc.tile_pool(name="peq", bufs=8) as peq,
            tc.tile_pool(name="pey", bufs=6) as pey,
            tc.tile_pool(name="pst", bufs=8) as pst,
            tc.tile_pool(name="psE", bufs=6, space="PSUM") as psE,
        ):
            g_b = pec.tile([P, D], F32)
            b_b = pec.tile([P, D], F32)
            eps_t = pec.tile([P, 1], F32)
            nc.sync.dma_start(g_b, gv[:].partition_broadcast(P))
            nc.sync.dma_start(b_b, bv2[:].partition_broadcast(P))
            nc.vector.memset(eps_t, 1e-5)
            xo = XO
            for rt in range(NRT_O):
                qr = peq.tile([P, D], F32)
                nc.gpsimd.dma_start(qr, qres[rt * P : (rt + 1) * P, :])
                y = pey.tile([P, D], F32)
                for o2 in range(2):
                    ps = psE.tile([P, 512], F32)
                    for hp in range(NOT):
                        nc.tensor.matmul(
                            ps,
                            xo[hp][:, rt * P : (rt + 1) * P],
                            wo[hp][:, o2 * 512 : (o2 + 1) * 512],
                            start=(hp == 0),
                            stop=(hp == NOT - 1),
                        )
                    nc.vector.tensor_add(
                        y[:, o2 * 512 : (o2 + 1) * 512],
                        ps,
                        qr[:, o2 * 512 : (o2 + 1) * 512],
                    )
                stats = pst.tile([P, 2, 6], F32)
                for sg in range(2):
                    nc.vector.bn_stats(
                        stats[:, sg, :], y[:, sg * 512 : (sg + 1) * 512]
                    )
                mv = pst.tile([P, 2], F32)
                nc.vector.bn_aggr(mv, stats)
                std = pst.tile([P, 1], F32)
                nc.scalar.activation(std, mv[:, 1:2], AF.Sqrt, bias=eps_t)
                rstd = pst.tile([P, 1], F32)
                nc.vector.reciprocal(rstd, std)
                nc.vector.tensor_scalar(
                    y,
                    y,
                    mv[:, 0:1],
                    rstd,
                    op0=ALU.subtract,
                    op1=ALU.mult,
                )
                eng = nc.vector if rt % 2 == 0 else nc.gpsimd
                eng.tensor_mul(y, y, g_b)
                eng.tensor_add(y, y, b_b)
                nc.sync.dma_start(out[rt * P : (rt + 1) * P, :], y)
        pwo_cm.__exit__(None, None, None)
        pxo_cm.__exit__(None, None, None)
    _split_sync_waits(nc)
    return nc


_NC = None


def _get_nc():
    global _NC
    if _NC is None:
        _NC = build_nc()
    return _NC


def _pack_dr(xT, dt):
    """[D, N] -> [D//256, 128, 2, N] DoubleRow-packed: feature g*256+j*128+p
    lands at [g, p, j]."""
    n = xT.shape[1]
    return np.ascontiguousarray(
        xT.reshape(D // 256, 2, P, n).transpose(0, 2, 1, 3).astype(dt)
    )


def prepare_in_maps(q, k, v, Wq, bq, Wk, bk, Wv, bv, Wo, bo, ln_g, ln_b):
    f = np.float32
    f8 = ml_dtypes.float8_e4m3
    q = np.asarray(q, f)
    k = np.asarray(k, f)
    v = np.asarray(v, f)
    scale = 1.0 / np.sqrt(np.float32(DK))
    WqT = _pack_dr(np.asarray(Wq, f).T * scale, f8)
    WkT = _pack_dr(np.asarray(Wk, f).T, f8)
    WvT = _pack_dr(np.asarray(Wv, f).T, f8)
    WoT = np.ascontiguousarray(np.asarray(Wo, f).T.astype(ml_dtypes.bfloat16))
    bq_s = np.asarray(bq, f) * scale
    # bv flows through attention unchanged (probs sum to 1), so its effect on
    # the O projection is the constant vector bv @ Wo^T — fold into qres.
    res_const = np.asarray(bo, f) + np.asarray(bv, f) @ np.asarray(Wo, f).T
    common = {
        "WqT": WqT,
        "WkT": WkT,
        "WvT": WvT,
        "WoT": WoT,
        "bq": bq_s,
        "ln_g": np.asarray(ln_g, f),
        "ln_b": np.asarray(ln_b, f),
        "onesv": np.ones((P, NRT_V * H), f8),
        "onesf": np.ones(DK, np.float32),
    }
    in_maps = []
    for c in range(8):
        b_, half = divmod(c, 2)
        qs = q[b_, half * M : (half + 1) * M, :]
        qres_c = qs + res_const[None, :]
        in_maps.append(
            dict(
                common,
                xqT=_pack_dr(np.ascontiguousarray(qs.T), f8),
                xkT=_pack_dr(np.ascontiguousarray(k[b_].T), f8),
                xvT=_pack_dr(np.ascontiguousarray(v[b_].T), f8),
                qres=np.ascontiguousarray(qres_c),
            )
        )
    return in_maps


def kernel(q, k, v, Wq, bq, Wk, bk, Wv, bv, Wo, bo, ln_g, ln_b):
    nc = _get_nc()
    in_maps = prepare_in_maps(q, k, v, Wq, bq, Wk, bk, Wv, bv, Wo, bo, ln_g, ln_b)
    res = run_bass_kernel_spmd(nc, in_maps, core_ids=list(range(8)))
    out = np.empty((B, S, D), np.float32)
    for c in range(8):
        b_, half = divmod(c, 2)
        out[b_, half * M : (half + 1) * M, :] = res.results[c]["out"]
    return out


# revision 21
# speedup vs baseline: 1.2684x; 1.0286x over previous
"""MultiHeadedAttention block (B=4, S=2048, D=1024, H=16) on 8 TRN2 cores.

Sharding: core c handles batch b=c//2 and query-row half c%2 (1024 rows).
Each core computes full K/V projections for its batch (2x redundant within a
batch pair), attention for all 16 heads over its 1024 query rows, then
O-projection + residual + LayerNorm. No collectives.

The real-HW bottleneck is the Activation engine's exp throughput
(~0.9 ns/elem, 33.5M softmax logits per core ~ 240 us). The attention inner
loop is software-pipelined in emission order so ACT never waits:
PE computes scores(k+1) while ACT does exp(k) while PE finishes PV(k-1),
with double-buffered score PSUM and a 4-deep exp-tile ring. K projection for
the next head pair is interleaved into the PE slack of the current one.

Device layouts (per core):
  Q^T  [o=1024, r=1024]  feature-major (partitions = features), per-ot tiles
  K^T  [o, k] projected per head pair inside the attention loop (no spill)
  V    [k=2048, o=1024]  row-major per-rt tiles, with a ones column per head
  scores computed transposed: S_t[k, q] = K_h^T Q_h  (softmax along k =
  partitions; exp without max-subtraction is safe: |logits| < ~3).
  P@V with the ones-augmented V gives the softmax denominator as row DK;
  normalization multiplies by a DMA-broadcast reciprocal.
Bias algebra: bk is dropped entirely (it shifts every logit of a softmax row
equally -> exactly cancels); bv and bo are folded into the residual tensor on
the host (exact by linearity of the O projection).
"""

import sys

if "/opt/trn_rl_repo" not in sys.path:
    sys.path.insert(0, "/opt/trn_rl_repo")

import ml_dtypes
import numpy as np

import concourse.bass as bass
import concourse.mybir as mybir
import concourse.tile as tile
from concourse.bass_utils import run_bass_kernel_spmd

B, S, D, H, DK = 4, 2048, 1024, 16, 64
P = 128
M = S // 2          # query rows per core
NDT = D // P        # 8 contraction chunks
NOT = D // P        # 8 output-feature chunks (= head pairs)
NHP = H // 2        # 8 head pairs
NKT = S // P        # 16 key chunks
NQT = M // 512      # 2 query 512-chunks
NRT_K = S // 512    # 4 key-row 512-chunks
NRT_V = S // P      # 16 V row chunks
NRT_O = M // P      # 8 output row chunks
F32 = mybir.dt.float32
MM_DT = mybir.dt.float32r
AF = mybir.ActivationFunctionType
ALU = mybir.AluOpType
BF16 = mybir.dt.bfloat16
F8 = mybir.dt.float8e4


def _split_sync_waits(nc, max_waits=1):
    """Split instructions carrying more than max_waits sem waits.

    The container's walrus rejects instructions with multiple sync wait
    commands, so excess waits move onto NoOp instructions inserted just
    before, on the same engine.
    """
    idx = 0
    for f in nc.m.functions:
        for blk in f.blocks:
            newl = []
            for inst in blk.instructions:
                si = inst.sync_info
                waits = list(si.on_wait) if si is not None and si.on_wait else []
                if len(waits) > max_waits:
                    extra = waits[max_waits:]
                    si.on_wait = waits[:max_waits]
                    for j in range(0, len(extra), max_waits):
                        nop = mybir.InstNoOp(name=f"I-wsplit-{idx}", ins=[], outs=[])
                        idx += 1
                        nop.engine = inst.engine
                        nop.sync_info = mybir.SyncInfo(
                            on_wait=extra[j : j + max_waits], on_update=[]
                        )
                        newl.append(nop)
                newl.append(inst)
            blk.instructions = newl


def build_nc(loops=0, unroll=1, ab=()):
    nc = bass.Bass()
    NDR = D // 256
    xqT = nc.dram_tensor("xqT", [NDR, P, 2, M], F8, kind="ExternalInput")
    xkT = nc.dram_tensor("xkT", [NDR, P, 2, S], F8, kind="ExternalInput")
    xvT = nc.dram_tensor("xvT", [NDR, P, 2, S], F8, kind="ExternalInput")
    qres = nc.dram_tensor("qres", [M, D], BF16, kind="ExternalInput")
    WqT = nc.dram_tensor("WqT", [NDR, P, 2, D], F8, kind="ExternalInput")
    WkT = nc.dram_tensor("WkT", [NDR, P, 2, D], F8, kind="ExternalInput")
    WvT = nc.dram_tensor("WvT", [NDR, P, 2, D], F8, kind="ExternalInput")
    WoT = nc.dram_tensor("WoT", [D, D], BF16, kind="ExternalInput")
    bqv = nc.dram_tensor("bq", [D], F32, kind="ExternalInput")
    gv = nc.dram_tensor("ln_g", [D], F32, kind="ExternalInput")
    bv2 = nc.dram_tensor("ln_b", [D], F32, kind="ExternalInput")
    ones8 = nc.dram_tensor("onesv", [P, NRT_V * H], F8, kind="ExternalInput")
    onesf = nc.dram_tensor("onesf", [DK], F32, kind="ExternalInput")
    out = nc.dram_tensor("out", [M, D], F32, kind="ExternalOutput")

    WoT_r = WoT[:, :].rearrange("(a p) o -> p a o", p=P)

    with tile.TileContext(nc) as tc:
      for _rep in range(max(1, unroll)):
        pxo_cm = tc.tile_pool(name="pxo", bufs=1)
        pxo = pxo_cm.__enter__()
        with (
            tc.tile_pool(name="pqv", bufs=1) as pqv,
        ):
            XO = [
                pxo.tile([P, M], BF16, tag=f"XO{i}", name=f"XO{i}")
                for i in range(NHP)
            ]
            if "noattn" in ab:
                for t in XO:
                    nc.vector.memset(t, 0.001)
            exf = None
            if "noexp" in ab:
                exf = pxo.tile([P, 2, 512], F8, tag="exf", name="exf")
                nc.vector.memset(exf, 0.001)

            QT = []
            for ot in range(NOT):
                t = pqv.tile([P, M], BF16, tag=f"QT{ot}", name=f"QT{ot}")
                QT.append(t)
            bq_p = pqv.tile([P, NOT], F32)
            nc.gpsimd.dma_start(bq_p, bqv[:].rearrange("(a p) -> p a", p=P))
            ones_t = pqv.tile([1, DK], MM_DT)
            nc.gpsimd.dma_start(
                ones_t, onesf[:].partition_broadcast(1).bitcast(MM_DT)
            )
            Vp = []
            for rtp in range(NRT_V // 2):
                t = pqv.tile(
                    [P, 2, H, DK + 1], F8, tag=f"Vp{rtp}", name=f"Vp{rtp}"
                )
                for j in range(2):
                    nc.gpsimd.dma_start(
                        t[:, j, :, DK : DK + 1],
                        ones8[:, (2 * rtp + j) * H : (2 * rtp + j + 1) * H],
                    )
                Vp.append(t)
            # wv loads early so phase B starts without a DMA stall
            pwv_cm = tc.tile_pool(name="pwv", bufs=1, side="right")
            pwv = pwv_cm.__enter__()
            wv = []
            for g in range(NDR):
                w_t = pwv.tile([P, 2, D], F8, tag=f"wv{g}", name=f"wv{g}")
                nc.gpsimd.dma_start(w_t, WvT[g, :, :, :])
                wv.append(w_t)

            pbx_cm = tc.tile_pool(name="pbx", bufs=1, side="right")
            pbx = pbx_cm.__enter__()
            psAB_cm = tc.tile_pool(name="psAB", bufs=8, space="PSUM")
            psAB = psAB_cm.__enter__()

            # ---- Phase A: Q^T = (Wq/8) @ x_q^T + bq/8, layout [o, r]
            with (
                tc.tile_pool(name="pa", bufs=1) as pa,
            ):
                wq = []
                xq = []
                for g in range(NDR):
                    w_t = pa.tile([P, 2, D], F8, tag=f"wq{g}", name=f"wq{g}")
                    nc.sync.dma_start(w_t, WqT[g, :, :, :])
                    wq.append(w_t)
                    x_t = pa.tile([P, 2, M], F8, tag=f"xq{g}", name=f"xq{g}")
                    nc.sync.dma_start(x_t, xqT[g, :, :, :])
                    xq.append(x_t)
                xv = []
                for g in range(NDR):
                    x_t = pbx.tile([P, 2, S], F8, tag=f"xv{g}", name=f"xv{g}")
                    nc.sync.dma_start(x_t, xvT[g, :, :, :])
                    xv.append(x_t)
                for ot in range(NOT):
                    for qt in range(NQT):
                        ps = psAB.tile([P, 512], F32, tag='ps', name='ps')
                        for g in range(NDR):
                            nc.tensor.matmul(
                                ps,
                                wq[g][:, :, ot * P : (ot + 1) * P],
                                xq[g][:, :, qt * 512 : (qt + 1) * 512],
                                start=(g == 0),
                                stop=(g == NDR - 1),
                                perf_mode=mybir.MatmulPerfMode.DoubleRow,
                            )
                        nc.vector.tensor_scalar_add(
                            QT[ot][:, qt * 512 : (qt + 1) * 512],
                            ps,
                            bq_p[:, ot : ot + 1],
                        )

            # xk/wk load during phase B so phase D starts without a DMA stall
            pdx_cm = tc.tile_pool(name="pdx", bufs=1)
            pdx = pdx_cm.__enter__()
            xk = []
            wk = []
            for g in range(NDR):
                x_t = pdx.tile([P, 2, S], F8, tag=f"xk{g}", name=f"xk{g}")
                nc.gpsimd.dma_start(x_t, xkT[g, :, :, :])
                xk.append(x_t)
            for g in range(NDR):
                w_t = pdx.tile([P, 2, D], F8, tag=f"wk{g}", name=f"wk{g}")
                nc.gpsimd.dma_start(w_t, WkT[g, :, :, :])
                wk.append(w_t)

            # ---- Phase B: V = x_v @ Wv^T (bias folded into qres), [r, o] fp8 pairs
            for rt in range(NRT_V):
                for o2 in range(2):
                    ps = psAB.tile([P, 512], F32, tag='ps', name='ps')
                    for g in range(NDR):
                        nc.tensor.matmul(
                            ps,
                            xv[g][:, :, rt * P : (rt + 1) * P],
                            wv[g][:, :, o2 * 512 : (o2 + 1) * 512],
                            start=(g == 0),
                            stop=(g == NDR - 1),
                            perf_mode=mybir.MatmulPerfMode.DoubleRow,
                        )
                    nc.vector.tensor_copy(
                        Vp[rt // 2][:, rt % 2, o2 * 8 : (o2 + 1) * 8, 0:DK],
                        ps[:, :].rearrange("p (h e) -> p h e", e=DK),
                    )

            pbx_cm.__exit__(None, None, None)
            pwv_cm.__exit__(None, None, None)
            psAB_cm.__exit__(None, None, None)

            # wo + E-state prefetch during D so phase E work can interleave
            pwo_cm = tc.tile_pool(name="pwo", bufs=NDT, side="right")
            pwo = pwo_cm.__enter__()
            wo = []
            for dt in range(NDT):
                w_t = pwo.tile([P, D], BF16, tag="wo", name=f"wo{dt}")
                nc.gpsimd.dma_start(w_t, WoT_r[:, dt, :])
                wo.append(w_t)
            pec_cm = tc.tile_pool(name="pec", side="right", bufs=1)
            pec = pec_cm.__enter__()
            peq_cm = tc.tile_pool(name="peq", side="right", bufs=1)
            peq = peq_cm.__enter__()
            pey_cm = tc.tile_pool(name="pey", side="right", bufs=4)
            pey = pey_cm.__enter__()
            pst_cm = tc.tile_pool(name="pst", side="right", bufs=8)
            pst = pst_cm.__enter__()
            g_b = pec.tile([P, D], F32)
            b_b = pec.tile([P, D], F32)
            eps_t = pec.tile([P, 1], F32)
            nc.sync.dma_start(g_b, gv[:].partition_broadcast(P))
            nc.sync.dma_start(b_b, bv2[:].partition_broadcast(P))
            nc.vector.memset(eps_t, 1e-5)
            qrs = []
            for rt in range(NRT_O):
                qr = peq.tile([P, D], BF16, tag=f"qr{rt}", name=f"qr{rt}")
                nc.gpsimd.dma_start(qr, qres[rt * P : (rt + 1) * P, :])
                qrs.append(qr)
            e_state = {}

            def e_mm(rt, o2, pspool):
                if rt not in e_state:
                    e_state[rt] = {"y": pey.tile([P, D], F32, tag="y", name="y")}
                y = e_state[rt]["y"]
                ps = pspool.tile([P, 512], F32, tag="kps", name="eps")
                for hp in range(NOT):
                    nc.tensor.matmul(
                        ps,
                        XO[hp][:, rt * P : (rt + 1) * P],
                        wo[hp][:, o2 * 512 : (o2 + 1) * 512],
                        start=(hp == 0),
                        stop=(hp == NOT - 1),
                    )
                nc.vector.tensor_add(
                    y[:, o2 * 512 : (o2 + 1) * 512],
                    ps,
                    qrs[rt][:, o2 * 512 : (o2 + 1) * 512],
                )

            def e_stats(rt):
                st = e_state[rt]
                y = st["y"]
                stats = pst.tile([P, 2, 6], F32)
                for sg in range(2):
                    nc.vector.bn_stats(
                        stats[:, sg, :], y[:, sg * 512 : (sg + 1) * 512]
                    )
                mv = pst.tile([P, 2], F32)
                nc.vector.bn_aggr(mv, stats)
                std = pst.tile([P, 1], F32)
                nc.scalar.activation(std, mv[:, 1:2], AF.Sqrt, bias=eps_t)
                rstd = pst.tile([P, 1], F32)
                nc.vector.reciprocal(rstd, std)
                st["mv"] = mv
                st["rstd"] = rstd

            def e_final(rt):
                st = e_state.pop(rt)
                y = st["y"]
                nc.vector.tensor_scalar(
                    y,
                    y,
                    st["mv"][:, 0:1],
                    st["rstd"],
                    op0=ALU.subtract,
                    op1=ALU.mult,
                )
                eng = nc.vector if rt % 2 == 0 else nc.gpsimd
                eng.tensor_mul(y, y, g_b)
                eng.tensor_add(y, y, b_b)
                nc.sync.dma_start(out[rt * P : (rt + 1) * P, :], y)

            # ---- Phase D: K^T projection fused with attention, per head pair.
            # Software-pipelined per (qt, h01) section of 8 KG=2 groups:
            # PE scores(g+1) || ACT exp(g) || PE pv(g-1). PV PSUM and score
            # PSUM are double-buffered so section boundaries don't stall, and
            # the normalize chain is deferred into the next section's slack.
            KG = 2
            NG = NKT // KG
            with (
                tc.tile_pool(name="pdkt", bufs=1) as pdkt,
                tc.tile_pool(name="pde", bufs=4) as pde,
                tc.tile_pool(name="pdr", bufs=2) as pdr,
                tc.tile_pool(name="psS", bufs=2, space="PSUM") as psS,
                tc.tile_pool(name="psK", bufs=1, space="PSUM") as psK,
                tc.tile_pool(name="psPV", bufs=3, space="PSUM") as psPV,
            ):
                kts = {}
                pending = []
                secno = [0]

                def flush_pending(force=False):
                    while pending and (force or pending[0][0] <= secno[0]):
                        pending.pop(0)[1]()

                def kproj_start(hp):
                    kts[hp] = pdkt.tile(
                        [P, S], BF16, tag=f"kt{hp}", name=f"kt{hp}"
                    )

                def kproj_chunk(hp, rt):
                    kt_t = kts[hp]
                    ps = psK.tile([P, 512], F32, tag="kps", name="kps")
                    for g in range(NDR):
                        nc.tensor.matmul(
                            ps,
                            wk[g][:, :, hp * P : (hp + 1) * P],
                            xk[g][:, :, rt * 512 : (rt + 1) * 512],
                            start=(g == 0),
                            stop=(g == NDR - 1),
                            perf_mode=mybir.MatmulPerfMode.DoubleRow,
                        )
                    nc.vector.tensor_copy(
                        kt_t[:, rt * 512 : (rt + 1) * 512], ps
                    )

                def section(hp, qt, h01, kp_slots):
                    kt_t = kts[hp]
                    head = 2 * hp + h01
                    pb_ = h01 * DK
                    qsl = slice(qt * 512, (qt + 1) * 512)
                    pv = psPV.tile([DK + 1, 512], F32, tag="pv", name="pv")
                    prev = None
                    for g in range(NG):
                        ss = psS.tile([P, KG, 512], F32, tag="ss", name="ss")
                        for j in range(KG):
                            kt = g * KG + j
                            nc.tensor.matmul(
                                ss[:, j, :],
                                kt_t[pb_ : pb_ + DK, kt * P : (kt + 1) * P],
                                QT[hp][pb_ : pb_ + DK, qsl],
                                start=True,
                                stop=True,
                                tile_position=(pb_, 0),
                            )
                        if g == 1:
                            flush_pending(force=True)
                        if g in kp_slots:
                            kp_slots[g]()
                        if "noexp" in ab:
                            ex = exf
                        else:
                            ex = pde.tile(
                                [P, KG, 512], F8, tag="ex", name="ex"
                            )
                            nc.scalar.activation(ex, ss, AF.Exp)
                        if prev is not None:
                            pex, pg = prev
                            nc.tensor.matmul(
                                pv,
                                Vp[pg][:, :, head, :],
                                pex,
                                start=(pg == 0),
                                stop=False,
                                perf_mode=mybir.MatmulPerfMode.DoubleRow,
                            )
                        prev = (ex, g)
                    pex, pg = prev
                    nc.tensor.matmul(
                        pv,
                        Vp[pg][:, :, head, :],
                        pex,
                        start=False,
                        stop=(pg == NG - 1),
                        perf_mode=mybir.MatmulPerfMode.DoubleRow,
                    )
                    dst = XO[hp][pb_ : pb_ + DK, qsl]
                    if "nonorm" in ab:
                        nc.vector.tensor_copy(dst, pv[0:DK, :])
                        return
                    rc = pdr.tile([1, 512], MM_DT, tag="rc", name="rc")
                    with nc.allow_low_precision(
                        reason="1/denom feeds f32r broadcast matmul"
                    ):
                        nc.vector.reciprocal(rc, pv[DK : DK + 1, :])

                    def norm(pv=pv, rc=rc, dst=dst):
                        rbp = psK.tile([DK, 512], F32, tag="kps", name="rbp")
                        nc.tensor.matmul(rbp, ones_t, rc, start=True, stop=True)
                        nc.vector.tensor_copy(dst, pv[0:DK, :])
                        nc.vector.tensor_mul(dst, dst, rbp)

                    pending.append((secno[0] + 2, norm))
                    secno[0] += 1

                if "noattn" not in ab:
                    for hp in range(NHP):
                        kproj_start(hp)
                    for rt in range(NRT_K):
                        kproj_chunk(0, rt)
                    for hp in range(NHP):
                        for h01 in range(2):
                            kp = {}
                            if hp + 1 < NHP:
                                for g, rt_ in ((3, h01 * 2), (6, h01 * 2 + 1)):
                                    kp[g] = (
                                        lambda hp=hp, rt=rt_: kproj_chunk(
                                            hp + 1, rt
                                        )
                                    )
                            section(hp, 0, h01, kp)
                    sec = 0
                    for hp in range(NHP):
                        for h01 in range(2):
                            rt, part = divmod(sec, 4)
                            if part == 0:
                                work = lambda rt=rt: e_mm(rt, 0, psK)
                            elif part == 1:
                                work = lambda rt=rt: e_mm(rt, 1, psK)
                            elif part == 2:
                                work = lambda rt=rt: e_stats(rt)
                            else:
                                work = lambda rt=rt: e_final(rt)
                            section(hp, 1, h01, {3: work})
                            sec += 1
                    flush_pending(force=True)

            pdx_cm.__exit__(None, None, None)

        # ---- Phase E tail: remaining output rows (rt 4..7)
        with (
            tc.tile_pool(name="psE", bufs=4, space="PSUM") as psE,
        ):
            start_rt = 0 if "noattn" in ab else NRT_O // 2
            for rt in range(start_rt, NRT_O):
                e_mm(rt, 0, psE)
                e_mm(rt, 1, psE)
                e_stats(rt)
                e_final(rt)
        pst_cm.__exit__(None, None, None)
        pey_cm.__exit__(None, None, None)
        peq_cm.__exit__(None, None, None)
        pec_cm.__exit__(None, None, None)
        pwo_cm.__exit__(None, None, None)
        pxo_cm.__exit__(None, None, None)
    _split_sync_waits(nc)
    return nc


_NC = None


def _get_nc():
    global _NC
    if _NC is None:
        _NC = build_nc()
    return _NC


def _pack_dr(xT, dt):
    """[D, N] -> [D//256, 128, 2, N] DoubleRow-packed: feature g*256+j*128+p
    lands at [g, p, j]."""
    n = xT.shape[1]
    return np.ascontiguousarray(
        xT.reshape(D // 256, 2, P, n).transpose(0, 2, 1, 3).astype(dt)
    )


def prepare_in_maps(q, k, v, Wq, bq, Wk, bk, Wv, bv, Wo, bo, ln_g, ln_b):
    f = np.float32
    f8 = ml_dtypes.float8_e4m3
    q = np.asarray(q, f)
    k = np.asarray(k, f)
    v = np.asarray(v, f)
    scale = 1.0 / np.sqrt(np.float32(DK))
    WqT = _pack_dr(np.asarray(Wq, f).T * scale, f8)
    WkT = _pack_dr(np.asarray(Wk, f).T, f8)
    WvT = _pack_dr(np.asarray(Wv, f).T, f8)
    WoT = np.ascontiguousarray(np.asarray(Wo, f).T.astype(ml_dtypes.bfloat16))
    bq_s = np.asarray(bq, f) * scale
    # bv flows through attention unchanged (probs sum to 1), so its effect on
    # the O projection is the constant vector bv @ Wo^T — fold into qres.
    res_const = np.asarray(bo, f) + np.asarray(bv, f) @ np.asarray(Wo, f).T
    common = {
        "WqT": WqT,
        "WkT": WkT,
        "WvT": WvT,
        "WoT": WoT,
        "bq": bq_s,
        "ln_g": np.asarray(ln_g, f),
        "ln_b": np.asarray(ln_b, f),
        "onesv": np.ones((P, NRT_V * H), f8),
        "onesf": np.ones(DK, np.float32),
    }
    in_maps = []
    for c in range(8):
        b_, half = divmod(c, 2)
        qs = q[b_, half * M : (half + 1) * M, :]
        qres_c = qs + res_const[None, :]
        in_maps.append(
            dict(
                common,
                xqT=_pack_dr(np.ascontiguousarray(qs.T), f8),
                xkT=_pack_dr(np.ascontiguousarray(k[b_].T), f8),
                xvT=_pack_dr(np.ascontiguousarray(v[b_].T), f8),
                qres=np.ascontiguousarray(qres_c.astype(ml_dtypes.bfloat16)),
            )
        )
    return in_maps


def kernel(q, k, v, Wq, bq, Wk, bk, Wv, bv, Wo, bo, ln_g, ln_b):
    nc = _get_nc()
    in_maps = prepare_in_maps(q, k, v, Wq, bq, Wk, bk, Wv, bv, Wo, bo, ln_g, ln_b)
    res = run_bass_kernel_spmd(nc, in_maps, core_ids=list(range(8)))
    out = np.empty((B, S, D), np.float32)
    for c in range(8):
        b_, half = divmod(c, 2)
        out[b_, half * M : (half + 1) * M, :] = res.results[c]["out"]
    return out


# revision 22
# speedup vs baseline: 1.2816x; 1.0104x over previous
"""MultiHeadedAttention block (B=4, S=2048, D=1024, H=16) on 8 TRN2 cores.

Sharding: core c handles batch b=c//2 and query-row half c%2 (1024 rows).
Each core computes full K/V projections for its batch (2x redundant within a
batch pair), attention for all 16 heads over its 1024 query rows, then
O-projection + residual + LayerNorm. No collectives.

The real-HW bottleneck is the Activation engine's exp throughput
(~0.9 ns/elem, 33.5M softmax logits per core ~ 240 us). The attention inner
loop is software-pipelined in emission order so ACT never waits:
PE computes scores(k+1) while ACT does exp(k) while PE finishes PV(k-1),
with double-buffered score PSUM and a 4-deep exp-tile ring. K projection for
the next head pair is interleaved into the PE slack of the current one.

Device layouts (per core):
  Q^T  [o=1024, r=1024]  feature-major (partitions = features), per-ot tiles
  K^T  [o, k] projected per head pair inside the attention loop (no spill)
  V    [k=2048, o=1024]  row-major per-rt tiles, with a ones column per head
  scores computed transposed: S_t[k, q] = K_h^T Q_h  (softmax along k =
  partitions; exp without max-subtraction is safe: |logits| < ~3).
  P@V with the ones-augmented V gives the softmax denominator as row DK;
  normalization multiplies by a DMA-broadcast reciprocal.
Bias algebra: bk is dropped entirely (it shifts every logit of a softmax row
equally -> exactly cancels); bv and bo are folded into the residual tensor on
the host (exact by linearity of the O projection).
"""

import sys

if "/opt/trn_rl_repo" not in sys.path:
    sys.path.insert(0, "/opt/trn_rl_repo")

import ml_dtypes
import numpy as np

import concourse.bass as bass
import concourse.mybir as mybir
import concourse.tile as tile
from concourse.bass_utils import run_bass_kernel_spmd

B, S, D, H, DK = 4, 2048, 1024, 16, 64
P = 128
M = S // 2          # query rows per core
NDT = D // P        # 8 contraction chunks
NOT = D // P        # 8 output-feature chunks (= head pairs)
NHP = H // 2        # 8 head pairs
NKT = S // P        # 16 key chunks
NQT = M // 512      # 2 query 512-chunks
NRT_K = S // 512    # 4 key-row 512-chunks
NRT_V = S // P      # 16 V row chunks
NRT_O = M // P      # 8 output row chunks
F32 = mybir.dt.float32
MM_DT = mybir.dt.float32r
AF = mybir.ActivationFunctionType
ALU = mybir.AluOpType
BF16 = mybir.dt.bfloat16
F8 = mybir.dt.float8e4


def _split_sync_waits(nc, max_waits=1):
    """Split instructions carrying more than max_waits sem waits.

    The container's walrus rejects instructions with multiple sync wait
    commands, so excess waits move onto NoOp instructions inserted just
    before, on the same engine.
    """
    idx = 0
    for f in nc.m.functions:
        for blk in f.blocks:
            newl = []
            for inst in blk.instructions:
                si = inst.sync_info
                waits = list(si.on_wait) if si is not None and si.on_wait else []
                if len(waits) > max_waits:
                    extra = waits[max_waits:]
                    si.on_wait = waits[:max_waits]
                    for j in range(0, len(extra), max_waits):
                        nop = mybir.InstNoOp(name=f"I-wsplit-{idx}", ins=[], outs=[])
                        idx += 1
                        nop.engine = inst.engine
                        nop.sync_info = mybir.SyncInfo(
                            on_wait=extra[j : j + max_waits], on_update=[]
                        )
                        newl.append(nop)
                newl.append(inst)
            blk.instructions = newl


def build_nc(loops=0, unroll=1, ab=()):
    nc = bass.Bass()
    NDR = D // 256
    xqT = nc.dram_tensor("xqT", [NDR, P, 2, M], F8, kind="ExternalInput")
    xkT = nc.dram_tensor("xkT", [NDR, P, 2, S], F8, kind="ExternalInput")
    xvT = nc.dram_tensor("xvT", [NDR, P, 2, S], F8, kind="ExternalInput")
    qres = nc.dram_tensor("qres", [M, D], BF16, kind="ExternalInput")
    WqT = nc.dram_tensor("WqT", [NDR, P, 2, D], F8, kind="ExternalInput")
    WkT = nc.dram_tensor("WkT", [NDR, P, 2, D], F8, kind="ExternalInput")
    WvT = nc.dram_tensor("WvT", [NDR, P, 2, D], F8, kind="ExternalInput")
    WoT = nc.dram_tensor("WoT", [D, D], BF16, kind="ExternalInput")
    bqv = nc.dram_tensor("bq", [D], F32, kind="ExternalInput")
    gv = nc.dram_tensor("ln_g", [D], F32, kind="ExternalInput")
    bv2 = nc.dram_tensor("ln_b", [D], F32, kind="ExternalInput")
    ones8 = nc.dram_tensor("onesv", [P, NRT_V * H], F8, kind="ExternalInput")
    onesf = nc.dram_tensor("onesf", [DK], F32, kind="ExternalInput")
    out = nc.dram_tensor("out", [M, D], F32, kind="ExternalOutput")

    WoT_r = WoT[:, :].rearrange("(a p) o -> p a o", p=P)

    with tile.TileContext(nc) as tc:
      for _rep in range(max(1, unroll)):
        pxo_cm = tc.tile_pool(name="pxo", bufs=1)
        pxo = pxo_cm.__enter__()
        with (
            tc.tile_pool(name="pqv", bufs=1) as pqv,
        ):
            XO = [
                pxo.tile([P, M], BF16, tag=f"XO{i}", name=f"XO{i}")
                for i in range(NHP)
            ]
            if "noattn" in ab:
                for t in XO:
                    nc.vector.memset(t, 0.001)
            exf = None
            if "noexp" in ab:
                exf = pxo.tile([P, 2, 512], F8, tag="exf", name="exf")
                nc.vector.memset(exf, 0.001)

            QT = []
            for ot in range(NOT):
                t = pqv.tile([P, M], BF16, tag=f"QT{ot}", name=f"QT{ot}")
                QT.append(t)
            bq_p = pqv.tile([P, NOT], F32)
            nc.gpsimd.dma_start(bq_p, bqv[:].rearrange("(a p) -> p a", p=P))
            ones_t = pqv.tile([1, DK], MM_DT)
            nc.gpsimd.dma_start(
                ones_t, onesf[:].partition_broadcast(1).bitcast(MM_DT)
            )
            Vp = []
            for rtp in range(NRT_V // 2):
                t = pqv.tile(
                    [P, 2, H, DK + 1], F8, tag=f"Vp{rtp}", name=f"Vp{rtp}"
                )
                for j in range(2):
                    nc.gpsimd.dma_start(
                        t[:, j, :, DK : DK + 1],
                        ones8[:, (2 * rtp + j) * H : (2 * rtp + j + 1) * H],
                    )
                Vp.append(t)
            # wv loads early so phase B starts without a DMA stall
            pwv_cm = tc.tile_pool(name="pwv", bufs=1, side="right")
            pwv = pwv_cm.__enter__()
            wv = []
            for g in range(NDR):
                w_t = pwv.tile([P, 2, D], F8, tag=f"wv{g}", name=f"wv{g}")
                nc.gpsimd.dma_start(w_t, WvT[g, :, :, :])
                wv.append(w_t)

            pbx_cm = tc.tile_pool(name="pbx", bufs=1, side="right")
            pbx = pbx_cm.__enter__()
            psAB_cm = tc.tile_pool(name="psAB", bufs=8, space="PSUM")
            psAB = psAB_cm.__enter__()

            # ---- Phase A: Q^T = (Wq/8) @ x_q^T + bq/8, layout [o, r]
            with (
                tc.tile_pool(name="pa", bufs=1) as pa,
            ):
                wq = []
                xq = []
                for g in range(NDR):
                    w_t = pa.tile([P, 2, D], F8, tag=f"wq{g}", name=f"wq{g}")
                    nc.sync.dma_start(w_t, WqT[g, :, :, :])
                    wq.append(w_t)
                    x_t = pa.tile([P, 2, M], F8, tag=f"xq{g}", name=f"xq{g}")
                    nc.sync.dma_start(x_t, xqT[g, :, :, :])
                    xq.append(x_t)
                xv = []
                for g in range(NDR):
                    x_t = pbx.tile([P, 2, S], F8, tag=f"xv{g}", name=f"xv{g}")
                    nc.sync.dma_start(x_t, xvT[g, :, :, :])
                    xv.append(x_t)
                for ot in range(NOT):
                    for qt in range(NQT):
                        ps = psAB.tile([P, 512], F32, tag='ps', name='ps')
                        for g in range(NDR):
                            nc.tensor.matmul(
                                ps,
                                wq[g][:, :, ot * P : (ot + 1) * P],
                                xq[g][:, :, qt * 512 : (qt + 1) * 512],
                                start=(g == 0),
                                stop=(g == NDR - 1),
                                perf_mode=mybir.MatmulPerfMode.DoubleRow,
                            )
                        nc.vector.tensor_scalar_add(
                            QT[ot][:, qt * 512 : (qt + 1) * 512],
                            ps,
                            bq_p[:, ot : ot + 1],
                        )

            # xk/wk load during phase B so phase D starts without a DMA stall
            pdx_cm = tc.tile_pool(name="pdx", bufs=1)
            pdx = pdx_cm.__enter__()
            xk = []
            wk = []
            for g in range(NDR):
                x_t = pdx.tile([P, 2, S], F8, tag=f"xk{g}", name=f"xk{g}")
                nc.gpsimd.dma_start(x_t, xkT[g, :, :, :])
                xk.append(x_t)
            for g in range(NDR):
                w_t = pdx.tile([P, 2, D], F8, tag=f"wk{g}", name=f"wk{g}")
                nc.gpsimd.dma_start(w_t, WkT[g, :, :, :])
                wk.append(w_t)

            # ---- Phase B: V = x_v @ Wv^T (bias folded into qres), [r, o] fp8 pairs
            for rt in range(NRT_V):
                for o2 in range(2):
                    ps = psAB.tile([P, 512], F32, tag='ps', name='ps')
                    for g in range(NDR):
                        nc.tensor.matmul(
                            ps,
                            xv[g][:, :, rt * P : (rt + 1) * P],
                            wv[g][:, :, o2 * 512 : (o2 + 1) * 512],
                            start=(g == 0),
                            stop=(g == NDR - 1),
                            perf_mode=mybir.MatmulPerfMode.DoubleRow,
                        )
                    nc.vector.tensor_copy(
                        Vp[rt // 2][:, rt % 2, o2 * 8 : (o2 + 1) * 8, 0:DK],
                        ps[:, :].rearrange("p (h e) -> p h e", e=DK),
                    )

            pbx_cm.__exit__(None, None, None)
            pwv_cm.__exit__(None, None, None)
            psAB_cm.__exit__(None, None, None)

            # wo + E-state prefetch during D so phase E work can interleave
            pwo_cm = tc.tile_pool(name="pwo", bufs=NDT, side="right")
            pwo = pwo_cm.__enter__()
            wo = []
            for dt in range(NDT):
                w_t = pwo.tile([P, D], BF16, tag="wo", name=f"wo{dt}")
                nc.gpsimd.dma_start(w_t, WoT_r[:, dt, :])
                wo.append(w_t)
            pec_cm = tc.tile_pool(name="pec", side="right", bufs=1)
            pec = pec_cm.__enter__()
            peq_cm = tc.tile_pool(name="peq", side="right", bufs=1)
            peq = peq_cm.__enter__()
            pey_cm = tc.tile_pool(name="pey", side="right", bufs=4)
            pey = pey_cm.__enter__()
            pst_cm = tc.tile_pool(name="pst", side="right", bufs=8)
            pst = pst_cm.__enter__()
            g_b = pec.tile([P, D], F32)
            b_b = pec.tile([P, D], F32)
            eps_t = pec.tile([P, 1], F32)
            nc.sync.dma_start(g_b, gv[:].partition_broadcast(P))
            nc.sync.dma_start(b_b, bv2[:].partition_broadcast(P))
            nc.vector.memset(eps_t, 1e-5)
            qrs = []
            for rt in range(NRT_O):
                qr = peq.tile([P, D], BF16, tag=f"qr{rt}", name=f"qr{rt}")
                nc.gpsimd.dma_start(qr, qres[rt * P : (rt + 1) * P, :])
                qrs.append(qr)
            e_state = {}

            def e_mm_half(rt, o2, half, pspool):
                st = e_state.setdefault(rt, {})
                if "y" not in st:
                    st["y"] = pey.tile([P, D], F32, tag="y", name="y")
                if half == 0:
                    st[f"ps{o2}"] = pspool.tile(
                        [P, 512], F32, tag="kps", name="eps"
                    )
                ps = st[f"ps{o2}"]
                for hp in range(half * 4, half * 4 + 4):
                    nc.tensor.matmul(
                        ps,
                        XO[hp][:, rt * P : (rt + 1) * P],
                        wo[hp][:, o2 * 512 : (o2 + 1) * 512],
                        start=(hp == 0),
                        stop=(hp == NOT - 1),
                    )
                if half == 1:
                    nc.vector.tensor_add(
                        st["y"][:, o2 * 512 : (o2 + 1) * 512],
                        ps,
                        qrs[rt][:, o2 * 512 : (o2 + 1) * 512],
                    )
                    del st[f"ps{o2}"]

            def e_mm(rt, o2, pspool):
                e_mm_half(rt, o2, 0, pspool)
                e_mm_half(rt, o2, 1, pspool)

            def e_stats(rt):
                st = e_state[rt]
                y = st["y"]
                stats = pst.tile([P, 2, 6], F32)
                for sg in range(2):
                    nc.vector.bn_stats(
                        stats[:, sg, :], y[:, sg * 512 : (sg + 1) * 512]
                    )
                mv = pst.tile([P, 2], F32)
                nc.vector.bn_aggr(mv, stats)
                std = pst.tile([P, 1], F32)
                nc.scalar.activation(std, mv[:, 1:2], AF.Sqrt, bias=eps_t)
                rstd = pst.tile([P, 1], F32)
                nc.vector.reciprocal(rstd, std)
                st["mv"] = mv
                st["rstd"] = rstd

            def e_final(rt):
                st = e_state.pop(rt)
                y = st["y"]
                nc.vector.tensor_scalar(
                    y,
                    y,
                    st["mv"][:, 0:1],
                    st["rstd"],
                    op0=ALU.subtract,
                    op1=ALU.mult,
                )
                eng = nc.vector if rt % 2 == 0 else nc.gpsimd
                eng.tensor_mul(y, y, g_b)
                eng.tensor_add(y, y, b_b)
                nc.sync.dma_start(out[rt * P : (rt + 1) * P, :], y)

            # ---- Phase D: K^T projection fused with attention, per head pair.
            # Software-pipelined per (qt, h01) section of 8 KG=2 groups:
            # PE scores(g+1) || ACT exp(g) || PE pv(g-1). PV PSUM and score
            # PSUM are double-buffered so section boundaries don't stall, and
            # the normalize chain is deferred into the next section's slack.
            KG = 2
            NG = NKT // KG
            with (
                tc.tile_pool(name="pdkt", bufs=1) as pdkt,
                tc.tile_pool(name="pde", bufs=4) as pde,
                tc.tile_pool(name="pdr", bufs=2) as pdr,
                tc.tile_pool(name="psS", bufs=2, space="PSUM") as psS,
                tc.tile_pool(name="psK", bufs=1, space="PSUM") as psK,
                tc.tile_pool(name="psR", bufs=1, space="PSUM") as psR,
                tc.tile_pool(name="psPV", bufs=2, space="PSUM") as psPV,
            ):
                kts = {}
                pending = []
                secno = [0]

                def flush_pending(g=99, force=False):
                    cur = (secno[0], g)
                    while pending and (force or pending[0][0] <= cur):
                        pending.pop(0)[1]()

                def kproj_start(hp):
                    kts[hp] = pdkt.tile(
                        [P, S], BF16, tag=f"kt{hp}", name=f"kt{hp}"
                    )

                def kproj_chunk(hp, rt):
                    kt_t = kts[hp]
                    ps = psK.tile([P, 512], F32, tag="kps", name="kps")
                    for g in range(NDR):
                        nc.tensor.matmul(
                            ps,
                            wk[g][:, :, hp * P : (hp + 1) * P],
                            xk[g][:, :, rt * 512 : (rt + 1) * 512],
                            start=(g == 0),
                            stop=(g == NDR - 1),
                            perf_mode=mybir.MatmulPerfMode.DoubleRow,
                        )
                    nc.vector.tensor_copy(
                        kt_t[:, rt * 512 : (rt + 1) * 512], ps
                    )

                def section(hp, qt, h01, kp_slots):
                    kt_t = kts[hp]
                    head = 2 * hp + h01
                    pb_ = h01 * DK
                    qsl = slice(qt * 512, (qt + 1) * 512)
                    pv = psPV.tile([DK + 1, 512], F32, tag="pv", name="pv")
                    prev = None
                    for g in range(NG):
                        ss = psS.tile([P, KG, 512], F32, tag="ss", name="ss")
                        for j in range(KG):
                            kt = g * KG + j
                            nc.tensor.matmul(
                                ss[:, j, :],
                                kt_t[pb_ : pb_ + DK, kt * P : (kt + 1) * P],
                                QT[hp][pb_ : pb_ + DK, qsl],
                                start=True,
                                stop=True,
                                tile_position=(pb_, 0),
                            )
                        if g == 1:
                            flush_pending(force=True)
                        if g in kp_slots:
                            kp_slots[g]()
                        if "noexp" in ab:
                            ex = exf
                        else:
                            ex = pde.tile(
                                [P, KG, 512], F8, tag="ex", name="ex"
                            )
                            nc.scalar.activation(ex, ss, AF.Exp)
                        if prev is not None:
                            pex, pg = prev
                            nc.tensor.matmul(
                                pv,
                                Vp[pg][:, :, head, :],
                                pex,
                                start=(pg == 0),
                                stop=False,
                                perf_mode=mybir.MatmulPerfMode.DoubleRow,
                            )
                        prev = (ex, g)
                    pex, pg = prev
                    nc.tensor.matmul(
                        pv,
                        Vp[pg][:, :, head, :],
                        pex,
                        start=False,
                        stop=(pg == NG - 1),
                        perf_mode=mybir.MatmulPerfMode.DoubleRow,
                    )
                    dst = XO[hp][pb_ : pb_ + DK, qsl]
                    if "nonorm" in ab:
                        nc.vector.tensor_copy(dst, pv[0:DK, :])
                        return
                    rc = pdr.tile([1, 512], MM_DT, tag="rc", name="rc")
                    with nc.allow_low_precision(
                        reason="1/denom feeds f32r broadcast matmul"
                    ):
                        nc.vector.reciprocal(rc, pv[DK : DK + 1, :])

                    def norm(pv=pv, rc=rc, dst=dst):
                        rbp = psR.tile([DK, 512], F32, tag="rbp", name="rbp")
                        nc.tensor.matmul(rbp, ones_t, rc, start=True, stop=True)
                        nc.vector.tensor_copy(dst, pv[0:DK, :])
                        nc.vector.tensor_mul(dst, dst, rbp)

                    pending.append(((secno[0] + 1, 4), norm))
                    secno[0] += 1

                if "noattn" not in ab:
                    for hp in range(NHP):
                        kproj_start(hp)
                    for rt in range(NRT_K):
                        kproj_chunk(0, rt)
                    for hp in range(NHP):
                        for h01 in range(2):
                            kp = {}
                            if hp + 1 < NHP:
                                for g, rt_ in ((3, h01 * 2), (6, h01 * 2 + 1)):
                                    kp[g] = (
                                        lambda hp=hp, rt=rt_: kproj_chunk(
                                            hp + 1, rt
                                        )
                                    )
                            section(hp, 0, h01, kp)
                    sec = 0
                    for hp in range(NHP):
                        for h01 in range(2):
                            rt, part = divmod(sec, 4)
                            if part in (0, 1):
                                slots = {
                                    3: lambda rt=rt, o2=part: e_mm_half(
                                        rt, o2, 0, psK
                                    ),
                                    6: lambda rt=rt, o2=part: e_mm_half(
                                        rt, o2, 1, psK
                                    ),
                                }
                            elif part == 2:
                                slots = {3: lambda rt=rt: e_stats(rt)}
                            else:
                                slots = {3: lambda rt=rt: e_final(rt)}
                            section(hp, 1, h01, slots)
                            sec += 1
                    flush_pending(force=True)

            pdx_cm.__exit__(None, None, None)

        # ---- Phase E tail: remaining output rows (rt 4..7)
        with (
            tc.tile_pool(name="psE", bufs=4, space="PSUM") as psE,
        ):
            start_rt = 0 if "noattn" in ab else NRT_O // 2
            for rt in range(start_rt, NRT_O):
                e_mm(rt, 0, psE)
                e_mm(rt, 1, psE)
                e_stats(rt)
                e_final(rt)
        pst_cm.__exit__(None, None, None)
        pey_cm.__exit__(None, None, None)
        peq_cm.__exit__(None, None, None)
        pec_cm.__exit__(None, None, None)
        pwo_cm.__exit__(None, None, None)
        pxo_cm.__exit__(None, None, None)
    _split_sync_waits(nc)
    return nc


_NC = None


def _get_nc():
    global _NC
    if _NC is None:
        _NC = build_nc()
    return _NC


def _pack_dr(xT, dt):
    """[D, N] -> [D//256, 128, 2, N] DoubleRow-packed: feature g*256+j*128+p
    lands at [g, p, j]."""
    n = xT.shape[1]
    return np.ascontiguousarray(
        xT.reshape(D // 256, 2, P, n).transpose(0, 2, 1, 3).astype(dt)
    )


def prepare_in_maps(q, k, v, Wq, bq, Wk, bk, Wv, bv, Wo, bo, ln_g, ln_b):
    f = np.float32
    f8 = ml_dtypes.float8_e4m3
    q = np.asarray(q, f)
    k = np.asarray(k, f)
    v = np.asarray(v, f)
    scale = 1.0 / np.sqrt(np.float32(DK))
    WqT = _pack_dr(np.asarray(Wq, f).T * scale, f8)
    WkT = _pack_dr(np.asarray(Wk, f).T, f8)
    WvT = _pack_dr(np.asarray(Wv, f).T, f8)
    WoT = np.ascontiguousarray(np.asarray(Wo, f).T.astype(ml_dtypes.bfloat16))
    bq_s = np.asarray(bq, f) * scale
    # bv flows through attention unchanged (probs sum to 1), so its effect on
    # the O projection is the constant vector bv @ Wo^T — fold into qres.
    res_const = np.asarray(bo, f) + np.asarray(bv, f) @ np.asarray(Wo, f).T
    common = {
        "WqT": WqT,
        "WkT": WkT,
        "WvT": WvT,
        "WoT": WoT,
        "bq": bq_s,
        "ln_g": np.asarray(ln_g, f),
        "ln_b": np.asarray(ln_b, f),
        "onesv": np.ones((P, NRT_V * H), f8),
        "onesf": np.ones(DK, np.float32),
    }
    in_maps = []
    for c in range(8):
        b_, half = divmod(c, 2)
        qs = q[b_, half * M : (half + 1) * M, :]
        qres_c = qs + res_const[None, :]
        in_maps.append(
            dict(
                common,
                xqT=_pack_dr(np.ascontiguousarray(qs.T), f8),
                xkT=_pack_dr(np.ascontiguousarray(k[b_].T), f8),
                xvT=_pack_dr(np.ascontiguousarray(v[b_].T), f8),
                qres=np.ascontiguousarray(qres_c.astype(ml_dtypes.bfloat16)),
            )
        )
    return in_maps


def kernel(q, k, v, Wq, bq, Wk, bk, Wv, bv, Wo, bo, ln_g, ln_b):
    nc = _get_nc()
    in_maps = prepare_in_maps(q, k, v, Wq, bq, Wk, bk, Wv, bv, Wo, bo, ln_g, ln_b)
    res = run_bass_kernel_spmd(nc, in_maps, core_ids=list(range(8)))
    out = np.empty((B, S, D), np.float32)
    for c in range(8):
        b_, half = divmod(c, 2)
        out[b_, half * M : (half + 1) * M, :] = res.results[c]["out"]
    return out


# revision 29
# speedup vs baseline: 1.3141x; 1.0253x over previous
"""MultiHeadedAttention block (B=4, S=2048, D=1024, H=16) on 8 TRN2 cores.

Sharding: core c handles batch b=c//2 and query-row half c%2 (1024 rows).
Each core computes full K/V projections for its batch (2x redundant within a
batch pair), attention for all 16 heads over its 1024 query rows, then
O-projection + residual + LayerNorm. No collectives.

The real-HW bottleneck is the Activation engine's exp throughput
(~0.9 ns/elem, 33.5M softmax logits per core ~ 240 us). The attention inner
loop is software-pipelined in emission order so ACT never waits:
PE computes scores(k+1) while ACT does exp(k) while PE finishes PV(k-1),
with double-buffered score PSUM and a 4-deep exp-tile ring. K projection for
the next head pair is interleaved into the PE slack of the current one.

Device layouts (per core):
  Q^T  [o=1024, r=1024]  feature-major (partitions = features), per-ot tiles
  K^T  [o, k] projected per head pair inside the attention loop (no spill)
  V    [k=2048, o=1024]  row-major per-rt tiles, with a ones column per head
  scores computed transposed: S_t[k, q] = K_h^T Q_h  (softmax along k =
  partitions; exp without max-subtraction is safe: |logits| < ~3).
  P@V with the ones-augmented V gives the softmax denominator as row DK;
  normalization multiplies by a DMA-broadcast reciprocal.
Bias algebra: bk is dropped entirely (it shifts every logit of a softmax row
equally -> exactly cancels); bv and bo are folded into the residual tensor on
the host (exact by linearity of the O projection).
"""

import sys

if "/opt/trn_rl_repo" not in sys.path:
    sys.path.insert(0, "/opt/trn_rl_repo")

import ml_dtypes
import numpy as np

import concourse.bass as bass
import concourse.mybir as mybir
import concourse.tile as tile
from concourse.bass_utils import run_bass_kernel_spmd

B, S, D, H, DK = 4, 2048, 1024, 16, 64
P = 128
M = S // 2          # query rows per core
NDT = D // P        # 8 contraction chunks
NOT = D // P        # 8 output-feature chunks (= head pairs)
NHP = H // 2        # 8 head pairs
NKT = S // P        # 16 key chunks
NQT = M // 512      # 2 query 512-chunks
NRT_K = S // 512    # 4 key-row 512-chunks
NRT_V = S // P      # 16 V row chunks
NRT_O = M // P      # 8 output row chunks
F32 = mybir.dt.float32
MM_DT = mybir.dt.float32r
AF = mybir.ActivationFunctionType
ALU = mybir.AluOpType
BF16 = mybir.dt.bfloat16
F8 = mybir.dt.float8e4


def _split_sync_waits(nc, max_waits=1):
    """Split instructions carrying more than max_waits sem waits.

    The container's walrus rejects instructions with multiple sync wait
    commands, so excess waits move onto NoOp instructions inserted just
    before, on the same engine.
    """
    idx = 0
    for f in nc.m.functions:
        for blk in f.blocks:
            newl = []
            for inst in blk.instructions:
                si = inst.sync_info
                waits = list(si.on_wait) if si is not None and si.on_wait else []
                if len(waits) > max_waits:
                    extra = waits[max_waits:]
                    si.on_wait = waits[:max_waits]
                    for j in range(0, len(extra), max_waits):
                        nop = mybir.InstNoOp(name=f"I-wsplit-{idx}", ins=[], outs=[])
                        idx += 1
                        nop.engine = inst.engine
                        nop.sync_info = mybir.SyncInfo(
                            on_wait=extra[j : j + max_waits], on_update=[]
                        )
                        newl.append(nop)
                newl.append(inst)
            blk.instructions = newl


def build_nc(loops=0, unroll=1, ab=()):
    nc = bass.Bass()
    NDR = D // 256
    xqT = nc.dram_tensor("xqT", [NDR, P, 2, M], F8, kind="ExternalInput")
    xkT = nc.dram_tensor("xkT", [NDR, P, 2, S], F8, kind="ExternalInput")
    xvT = nc.dram_tensor("xvT", [NDR, P, 2, S], F8, kind="ExternalInput")
    qres = nc.dram_tensor("qres", [M, D], BF16, kind="ExternalInput")
    WqT = nc.dram_tensor("WqT", [NDR, P, 2, D], F8, kind="ExternalInput")
    WkT = nc.dram_tensor("WkT", [NDR, P, 2, D], F8, kind="ExternalInput")
    WvT = nc.dram_tensor("WvT", [NDR, P, 2, D], F8, kind="ExternalInput")
    WoT = nc.dram_tensor("WoT", [D, D], BF16, kind="ExternalInput")
    bqv = nc.dram_tensor("bq", [D], F32, kind="ExternalInput")
    gv = nc.dram_tensor("ln_g", [D], F32, kind="ExternalInput")
    bv2 = nc.dram_tensor("ln_b", [D], F32, kind="ExternalInput")
    ones8 = nc.dram_tensor("onesv", [P, NRT_V * H], F8, kind="ExternalInput")
    onesf = nc.dram_tensor("onesf", [DK], F32, kind="ExternalInput")
    out = nc.dram_tensor("out", [M, D], F32, kind="ExternalOutput")

    WoT_r = WoT[:, :].rearrange("(a p) o -> p a o", p=P)

    with tile.TileContext(nc) as tc:
      for _rep in range(max(1, unroll)):
        pxo_cm = tc.tile_pool(name="pxo", bufs=1)
        pxo = pxo_cm.__enter__()
        with (
            tc.tile_pool(name="pqv", bufs=1) as pqv,
        ):
            XO = [
                pxo.tile([P, M], BF16, tag=f"XO{i}", name=f"XO{i}")
                for i in range(NHP)
            ]
            if "noattn" in ab:
                for t in XO:
                    nc.vector.memset(t, 0.001)
            exf = None
            if "noexp" in ab:
                exf = pxo.tile([P, 2, 512], F8, tag="exf", name="exf")
                nc.vector.memset(exf, 0.001)

            QT = []
            for ot in range(NOT):
                t = pqv.tile([P, M], BF16, tag=f"QT{ot}", name=f"QT{ot}")
                QT.append(t)
            bq_p = pqv.tile([P, NOT], F32)
            nc.gpsimd.dma_start(bq_p, bqv[:].rearrange("(a p) -> p a", p=P))
            ones_t = pqv.tile([1, DK], MM_DT)
            nc.gpsimd.dma_start(
                ones_t, onesf[:].partition_broadcast(1).bitcast(MM_DT)
            )
            Vp = []
            for rtp in range(NRT_V // 2):
                t = pqv.tile(
                    [P, 2, H, DK + 1], F8, tag=f"Vp{rtp}", name=f"Vp{rtp}"
                )
                for j in range(2):
                    nc.gpsimd.dma_start(
                        t[:, j, :, DK : DK + 1],
                        ones8[:, (2 * rtp + j) * H : (2 * rtp + j + 1) * H],
                    )
                Vp.append(t)
            # wv loads early so phase B starts without a DMA stall
            pwv_cm = tc.tile_pool(name="pwv", bufs=1)
            pwv = pwv_cm.__enter__()
            wv = []
            for g in range(NDR):
                w_t = pwv.tile([P, 2, D], F8, tag=f"wv{g}", name=f"wv{g}")
                nc.gpsimd.dma_start(w_t, WvT[g, :, :, :])
                wv.append(w_t)

            pbx_cm = tc.tile_pool(name="pbx", bufs=1)
            pbx = pbx_cm.__enter__()
            psX_cm = tc.tile_pool(name="psX", bufs=2, space="PSUM")
            psX = psX_cm.__enter__()

            # ---- Phase A: Q^T = (Wq/8) @ x_q^T + bq/8, layout [o, r]
            pa_cm = tc.tile_pool(name="pa", bufs=1)
            pa = pa_cm.__enter__()
            if True:
                wq = []
                xq = []
                for g in range(NDR):
                    w_t = pa.tile([P, 2, D], F8, tag=f"wq{g}", name=f"wq{g}")
                    nc.sync.dma_start(w_t, WqT[g, :, :, :])
                    wq.append(w_t)
                    x_t = pa.tile([P, 2, M], F8, tag=f"xq{g}", name=f"xq{g}")
                    nc.sync.dma_start(x_t, xqT[g, :, :, :])
                    xq.append(x_t)
                xv = []
                for g in range(NDR):
                    x_t = pbx.tile([P, 2, S], F8, tag=f"xv{g}", name=f"xv{g}")
                    nc.sync.dma_start(x_t, xvT[g, :, :, :])
                    xv.append(x_t)
                a_work = []

                def a_group(ot, qt):
                    ps = psX.tile([P, 512], F32, tag='px', name='px')
                    for g in range(NDR):
                        nc.tensor.matmul(
                            ps,
                            wq[g][:, :, ot * P : (ot + 1) * P],
                            xq[g][:, :, qt * 512 : (qt + 1) * 512],
                            start=(g == 0),
                            stop=(g == NDR - 1),
                            perf_mode=mybir.MatmulPerfMode.DoubleRow,
                        )
                    nc.vector.tensor_scalar_add(
                        QT[ot][:, qt * 512 : (qt + 1) * 512],
                        ps,
                        bq_p[:, ot : ot + 1],
                    )

                for qt in range(NQT):
                    a_group(0, qt)
                for ot in range(1, NOT):
                    for qt in range(NQT):
                        a_work.append(
                            lambda ot=ot, qt=qt: a_group(ot, qt)
                        )

            # xk/wk load during phase B so phase D starts without a DMA stall
            pdx_cm = tc.tile_pool(name="pdx", bufs=1)
            pdx = pdx_cm.__enter__()
            xk = []
            wk = []
            for g in range(NDR):
                x_t = pdx.tile([P, 2, S], F8, tag=f"xk{g}", name=f"xk{g}")
                nc.gpsimd.dma_start(x_t, xkT[g, :, :, :])
                xk.append(x_t)
            for g in range(NDR):
                w_t = pdx.tile([P, 2, D], F8, tag=f"wk{g}", name=f"wk{g}")
                nc.gpsimd.dma_start(w_t, WkT[g, :, :, :])
                wk.append(w_t)

            # ---- Phase B: V = x_v @ Wv^T (bias folded into qres), [r, o] fp8
            # pairs. First half runs as a phase (needed by the first PV
            # groups); second half is folded into the first attention section.
            b_work = []

            def b_group(rt, o2):
                ps = psX.tile([P, 512], F32, tag='px', name='px')
                for g in range(NDR):
                    nc.tensor.matmul(
                        ps,
                        xv[g][:, :, rt * P : (rt + 1) * P],
                        wv[g][:, :, o2 * 512 : (o2 + 1) * 512],
                        start=(g == 0),
                        stop=(g == NDR - 1),
                        perf_mode=mybir.MatmulPerfMode.DoubleRow,
                    )
                nc.vector.tensor_copy(
                    Vp[rt // 2][:, rt % 2, o2 * 8 : (o2 + 1) * 8, 0:DK],
                    ps[:, :].rearrange("p (h e) -> p h e", e=DK),
                )

            for rt in range(NRT_V // 2):
                for o2 in range(2):
                    b_group(rt, o2)
            for rt in range(NRT_V // 2, NRT_V):
                for o2 in range(2):
                    b_work.append(lambda rt=rt, o2=o2: b_group(rt, o2))

            # wo + E-state prefetch during D so phase E work can interleave
            pwo_cm = tc.tile_pool(name="pwo", bufs=NDT, side="right")
            pwo = pwo_cm.__enter__()
            wo = []
            for dt in range(NDT):
                w_t = pwo.tile([P, D], BF16, tag="wo", name=f"wo{dt}")
                nc.gpsimd.dma_start(w_t, WoT_r[:, dt, :])
                wo.append(w_t)
            pec_cm = tc.tile_pool(name="pec", side="right", bufs=1)
            pec = pec_cm.__enter__()
            peq_cm = tc.tile_pool(name="peq", side="right", bufs=1)
            peq = peq_cm.__enter__()
            pey_cm = tc.tile_pool(name="pey", side="right", bufs=4)
            pey = pey_cm.__enter__()
            pst_cm = tc.tile_pool(name="pst", side="right", bufs=8)
            pst = pst_cm.__enter__()
            g_b = pec.tile([P, D], F32)
            b_b = pec.tile([P, D], F32)
            eps_t = pec.tile([P, 1], F32)
            nc.sync.dma_start(g_b, gv[:].partition_broadcast(P))
            nc.sync.dma_start(b_b, bv2[:].partition_broadcast(P))
            nc.vector.memset(eps_t, 1e-5)
            qrs = []
            for rt in range(NRT_O):
                qr = peq.tile([P, D], BF16, tag=f"qr{rt}", name=f"qr{rt}")
                nc.gpsimd.dma_start(qr, qres[rt * P : (rt + 1) * P, :])
                qrs.append(qr)
            e_state = {}

            def e_mm_half(rt, o2, half, pspool):
                st = e_state.setdefault(rt, {})
                if "y" not in st:
                    st["y"] = pey.tile([P, D], F32, tag="y", name="y")
                if half == 0:
                    st[f"ps{o2}"] = pspool.tile(
                        [P, 512], F32, tag="px", name="eps"
                    )
                ps = st[f"ps{o2}"]
                for hp in range(half * 4, half * 4 + 4):
                    nc.tensor.matmul(
                        ps,
                        XO[hp][:, rt * P : (rt + 1) * P],
                        wo[hp][:, o2 * 512 : (o2 + 1) * 512],
                        start=(hp == 0),
                        stop=(hp == NOT - 1),
                    )
                if half == 1:
                    nc.vector.tensor_add(
                        st["y"][:, o2 * 512 : (o2 + 1) * 512],
                        ps,
                        qrs[rt][:, o2 * 512 : (o2 + 1) * 512],
                    )
                    del st[f"ps{o2}"]

            def e_mm(rt, o2, pspool):
                e_mm_half(rt, o2, 0, pspool)
                e_mm_half(rt, o2, 1, pspool)

            def e_stats(rt):
                st = e_state[rt]
                y = st["y"]
                stats = pst.tile([P, 2, 6], F32)
                for sg in range(2):
                    nc.vector.bn_stats(
                        stats[:, sg, :], y[:, sg * 512 : (sg + 1) * 512]
                    )
                mv = pst.tile([P, 2], F32)
                nc.vector.bn_aggr(mv, stats)
                std = pst.tile([P, 1], F32)
                nc.scalar.activation(std, mv[:, 1:2], AF.Sqrt, bias=eps_t)
                rstd = pst.tile([P, 1], F32)
                nc.vector.reciprocal(rstd, std)
                st["mv"] = mv
                st["rstd"] = rstd

            def e_final(rt):
                st = e_state.pop(rt)
                y = st["y"]
                eng = nc.vector if rt < NRT_O // 2 else nc.gpsimd
                eng.tensor_scalar(
                    y,
                    y,
                    st["mv"][:, 0:1],
                    st["rstd"],
                    op0=ALU.subtract,
                    op1=ALU.mult,
                )
                eng.tensor_mul(y, y, g_b)
                eng.tensor_add(y, y, b_b)
                nc.sync.dma_start(out[rt * P : (rt + 1) * P, :], y)

            # ---- Phase D: K^T projection fused with attention, per head pair.
            # Software-pipelined per (qt, h01) section of 8 KG=2 groups:
            # PE scores(g+1) || ACT exp(g) || PE pv(g-1). PV PSUM and score
            # PSUM are double-buffered so section boundaries don't stall, and
            # the normalize chain is deferred into the next section's slack.
            KG = 2
            NG = NKT // KG
            with (
                tc.tile_pool(name="pdkt", bufs=1) as pdkt,
                tc.tile_pool(name="pde", bufs=4) as pde,
                tc.tile_pool(name="pdr", bufs=2) as pdr,
                tc.tile_pool(name="psS", bufs=2, space="PSUM") as psS,
                tc.tile_pool(name="psPV", bufs=2, space="PSUM") as psPV,
            ):
                kts = {}
                pending = []
                secno = [0]

                def flush_pending(g=99, force=False):
                    cur = (secno[0], g)
                    while pending and (force or pending[0][0] <= cur):
                        pending.pop(0)[1]()

                def kproj_start(hp):
                    kts[hp] = pdkt.tile(
                        [P, S], F8, tag=f"kt{hp}", name=f"kt{hp}"
                    )

                def kproj_chunk(hp, rt):
                    kt_t = kts[hp]
                    ps = psX.tile([P, 512], F32, tag="px", name="kps")
                    for g in range(NDR):
                        nc.tensor.matmul(
                            ps,
                            wk[g][:, :, hp * P : (hp + 1) * P],
                            xk[g][:, :, rt * 512 : (rt + 1) * 512],
                            start=(g == 0),
                            stop=(g == NDR - 1),
                            perf_mode=mybir.MatmulPerfMode.DoubleRow,
                        )
                    nc.vector.tensor_copy(
                        kt_t[:, rt * 512 : (rt + 1) * 512], ps
                    )

                def section(hp, qt, h01, kp_slots):
                    kt_t = kts[hp]
                    head = 2 * hp + h01
                    pb_ = h01 * DK
                    qsl = slice(qt * 512, (qt + 1) * 512)
                    pv = psPV.tile([DK + 1, 512], F32, tag="pv", name="pv")
                    prev = None
                    for g in range(NG):
                        ss = psS.tile([P, KG, 512], F32, tag="ss", name="ss")
                        for j in range(KG):
                            kt = g * KG + j
                            nc.tensor.matmul(
                                ss[:, j, :],
                                kt_t[pb_ : pb_ + DK, kt * P : (kt + 1) * P],
                                QT[hp][pb_ : pb_ + DK, qsl],
                                start=True,
                                stop=True,
                                tile_position=(pb_, 0),
                            )
                        if g in (1, 4):
                            flush_pending(g)
                        if g in kp_slots:
                            kp_slots[g]()
                        if secno[0] == 0 and b_work:
                            b_work.pop(0)()
                            if b_work:
                                b_work.pop(0)()
                        elif secno[0] >= 1 and g in (2, 5) and a_work:
                            a_work.pop(0)()
                        if "noexp" in ab:
                            ex = exf
                        else:
                            ex = pde.tile(
                                [P, KG, 512], F8, tag="ex", name="ex"
                            )
                            nc.scalar.activation(ex, ss, AF.Exp)
                        if prev is not None:
                            pex, pg = prev
                            nc.tensor.matmul(
                                pv,
                                Vp[pg][:, :, head, :],
                                pex,
                                start=(pg == 0),
                                stop=False,
                                perf_mode=mybir.MatmulPerfMode.DoubleRow,
                            )
                        prev = (ex, g)
                    pex, pg = prev
                    nc.tensor.matmul(
                        pv,
                        Vp[pg][:, :, head, :],
                        pex,
                        start=False,
                        stop=(pg == NG - 1),
                        perf_mode=mybir.MatmulPerfMode.DoubleRow,
                    )
                    dst = XO[hp][pb_ : pb_ + DK, qsl]
                    if "nonorm" in ab:
                        nc.vector.tensor_copy(dst, pv[0:DK, :])
                        return
                    rc = pdr.tile([1, 512], MM_DT, tag="rc", name="rc")
                    with nc.allow_low_precision(
                        reason="1/denom feeds f32r broadcast matmul"
                    ):
                        nc.vector.reciprocal(rc, pv[DK : DK + 1, :])

                    def norm(pv=pv, rc=rc, dst=dst):
                        rbp = psX.tile([DK, 512], F32, tag="px", name="rbp")
                        nc.tensor.matmul(rbp, ones_t, rc, start=True, stop=True)
                        nc.vector.tensor_copy(dst, pv[0:DK, :])
                        nc.vector.tensor_mul(dst, dst, rbp)

                    pending.append(((secno[0] + 1, 4), norm))
                    secno[0] += 1

                if "noattn" not in ab:
                    for hp in range(NHP):
                        kproj_start(hp)
                    for rt in range(NRT_K):
                        kproj_chunk(0, rt)
                    for hp in range(NHP):
                        for h01 in range(2):
                            kp = {}
                            if hp + 1 < NHP:
                                for g, rt_ in ((3, h01 * 2), (6, h01 * 2 + 1)):
                                    kp[g] = (
                                        lambda hp=hp, rt=rt_: kproj_chunk(
                                            hp + 1, rt
                                        )
                                    )
                            section(hp, 0, h01, kp)
                    sec = 0
                    for hp in range(NHP):
                        for h01 in range(2):
                            rt, part = divmod(sec, 4)
                            if part in (0, 1):
                                slots = {
                                    3: lambda rt=rt, o2=part: e_mm_half(
                                        rt, o2, 0, psX
                                    ),
                                    6: lambda rt=rt, o2=part: e_mm_half(
                                        rt, o2, 1, psX
                                    ),
                                }
                            elif part == 2:
                                slots = {3: lambda rt=rt: e_stats(rt)}
                            else:
                                slots = {3: lambda rt=rt: e_final(rt)}
                            section(hp, 1, h01, slots)
                            sec += 1
                    flush_pending(force=True)

            pdx_cm.__exit__(None, None, None)
            pa_cm.__exit__(None, None, None)
            pbx_cm.__exit__(None, None, None)
            pwv_cm.__exit__(None, None, None)
            psX_cm.__exit__(None, None, None)

        # ---- Phase E tail: remaining output rows (rt 4..7)
        with (
            tc.tile_pool(name="psE", bufs=2, space="PSUM") as psE,
        ):
            start_rt = 0 if "noattn" in ab else NRT_O // 2
            for rt in range(start_rt, NRT_O):
                e_mm(rt, 0, psE)
                e_mm(rt, 1, psE)
                e_stats(rt)
                e_final(rt)
        pst_cm.__exit__(None, None, None)
        pey_cm.__exit__(None, None, None)
        peq_cm.__exit__(None, None, None)
        pec_cm.__exit__(None, None, None)
        pwo_cm.__exit__(None, None, None)
        pxo_cm.__exit__(None, None, None)
    _split_sync_waits(nc)
    return nc


_NC = None


def _get_nc():
    global _NC
    if _NC is None:
        _NC = build_nc()
    return _NC


def _pack_dr(xT, dt):
    """[D, N] -> [D//256, 128, 2, N] DoubleRow-packed: feature g*256+j*128+p
    lands at [g, p, j]."""
    n = xT.shape[1]
    return np.ascontiguousarray(
        xT.reshape(D // 256, 2, P, n).transpose(0, 2, 1, 3).astype(dt)
    )


def prepare_in_maps(q, k, v, Wq, bq, Wk, bk, Wv, bv, Wo, bo, ln_g, ln_b):
    f = np.float32
    f8 = ml_dtypes.float8_e4m3
    q = np.asarray(q, f)
    k = np.asarray(k, f)
    v = np.asarray(v, f)
    scale = 1.0 / np.sqrt(np.float32(DK))
    WqT = _pack_dr(np.asarray(Wq, f).T * scale, f8)
    WkT = _pack_dr(np.asarray(Wk, f).T, f8)
    WvT = _pack_dr(np.asarray(Wv, f).T, f8)
    WoT = np.ascontiguousarray(np.asarray(Wo, f).T.astype(ml_dtypes.bfloat16))
    bq_s = np.asarray(bq, f) * scale
    # bv flows through attention unchanged (probs sum to 1), so its effect on
    # the O projection is the constant vector bv @ Wo^T — fold into qres.
    res_const = np.asarray(bo, f) + np.asarray(bv, f) @ np.asarray(Wo, f).T
    common = {
        "WqT": WqT,
        "WkT": WkT,
        "WvT": WvT,
        "WoT": WoT,
        "bq": bq_s,
        "ln_g": np.asarray(ln_g, f),
        "ln_b": np.asarray(ln_b, f),
        "onesv": np.ones((P, NRT_V * H), f8),
        "onesf": np.ones(DK, np.float32),
    }
    in_maps = []
    for c in range(8):
        b_, half = divmod(c, 2)
        qs = q[b_, half * M : (half + 1) * M, :]
        qres_c = qs + res_const[None, :]
        in_maps.append(
            dict(
                common,
                xqT=_pack_dr(np.ascontiguousarray(qs.T), f8),
                xkT=_pack_dr(np.ascontiguousarray(k[b_].T), f8),
                xvT=_pack_dr(np.ascontiguousarray(v[b_].T), f8),
                qres=np.ascontiguousarray(qres_c.astype(ml_dtypes.bfloat16)),
            )
        )
    return in_maps


def kernel(q, k, v, Wq, bq, Wk, bk, Wv, bv, Wo, bo, ln_g, ln_b):
    nc = _get_nc()
    in_maps = prepare_in_maps(q, k, v, Wq, bq, Wk, bk, Wv, bv, Wo, bo, ln_g, ln_b)
    res = run_bass_kernel_spmd(nc, in_maps, core_ids=list(range(8)))
    out = np.empty((B, S, D), np.float32)
    for c in range(8):
        b_, half = divmod(c, 2)
        out[b_, half * M : (half + 1) * M, :] = res.results[c]["out"]
    return out


# revision 33
# speedup vs baseline: 1.3838x; 1.0531x over previous
"""MultiHeadedAttention block (B=4, S=2048, D=1024, H=16) on 8 TRN2 cores.

Sharding: core c handles batch b=c//2 and query-row half c%2 (1024 rows).
Each core computes full K/V projections for its batch (2x redundant within a
batch pair), attention for all 16 heads over its 1024 query rows, then
O-projection + residual + LayerNorm. No collectives.

The real-HW bottleneck is the Activation engine's exp throughput
(~0.9 ns/elem, 33.5M softmax logits per core ~ 240 us). The attention inner
loop is software-pipelined in emission order so ACT never waits:
PE computes scores(k+1) while ACT does exp(k) while PE finishes PV(k-1),
with double-buffered score PSUM and a 4-deep exp-tile ring. K projection for
the next head pair is interleaved into the PE slack of the current one.

Device layouts (per core):
  Q^T  [o=1024, r=1024]  feature-major (partitions = features), per-ot tiles
  K^T  [o, k] projected per head pair inside the attention loop (no spill)
  V    [k=2048, o=1024]  row-major per-rt tiles, with a ones column per head
  scores computed transposed: S_t[k, q] = K_h^T Q_h  (softmax along k =
  partitions; exp without max-subtraction is safe: |logits| < ~3).
  P@V with the ones-augmented V gives the softmax denominator as row DK;
  normalization multiplies by a DMA-broadcast reciprocal.
Bias algebra: bk is dropped entirely (it shifts every logit of a softmax row
equally -> exactly cancels); bv and bo are folded into the residual tensor on
the host (exact by linearity of the O projection).
"""

import sys

if "/opt/trn_rl_repo" not in sys.path:
    sys.path.insert(0, "/opt/trn_rl_repo")

import ml_dtypes
import numpy as np

import concourse.bass as bass
import concourse.mybir as mybir
import concourse.tile as tile
from concourse.bass_utils import run_bass_kernel_spmd

B, S, D, H, DK = 4, 2048, 1024, 16, 64
P = 128
M = S // 2          # query rows per core
NDT = D // P        # 8 contraction chunks
NOT = D // P        # 8 output-feature chunks (= head pairs)
NHP = H // 2        # 8 head pairs
NKT = S // P        # 16 key chunks
NQT = M // 512      # 2 query 512-chunks
NRT_K = S // 512    # 4 key-row 512-chunks
NRT_V = S // P      # 16 V row chunks
NRT_O = M // P      # 8 output row chunks
F32 = mybir.dt.float32
MM_DT = mybir.dt.float32r
AF = mybir.ActivationFunctionType
ALU = mybir.AluOpType
BF16 = mybir.dt.bfloat16
F8 = mybir.dt.float8e4


def _split_sync_waits(nc, max_waits=1):
    """Split instructions carrying more than max_waits sem waits.

    The container's walrus rejects instructions with multiple sync wait
    commands, so excess waits move onto NoOp instructions inserted just
    before, on the same engine.
    """
    idx = 0
    for f in nc.m.functions:
        for blk in f.blocks:
            newl = []
            for inst in blk.instructions:
                si = inst.sync_info
                waits = list(si.on_wait) if si is not None and si.on_wait else []
                if len(waits) > max_waits:
                    extra = waits[max_waits:]
                    si.on_wait = waits[:max_waits]
                    for j in range(0, len(extra), max_waits):
                        nop = mybir.InstNoOp(name=f"I-wsplit-{idx}", ins=[], outs=[])
                        idx += 1
                        nop.engine = inst.engine
                        nop.sync_info = mybir.SyncInfo(
                            on_wait=extra[j : j + max_waits], on_update=[]
                        )
                        newl.append(nop)
                newl.append(inst)
            blk.instructions = newl


def build_nc(loops=0, unroll=1, ab=()):
    nc = bass.Bass()
    NDR = D // 256
    xqT = nc.dram_tensor("xqT", [NDR, P, 2, M], F8, kind="ExternalInput")
    xkT = nc.dram_tensor("xkT", [NDR, P, 2, S], F8, kind="ExternalInput")
    xvT = nc.dram_tensor("xvT", [NDR, P, 2, S], F8, kind="ExternalInput")
    qres = nc.dram_tensor("qres", [M, D], BF16, kind="ExternalInput")
    WqT = nc.dram_tensor("WqT", [NDR, P, 2, D], F8, kind="ExternalInput")
    WkT = nc.dram_tensor("WkT", [NDR, P, 2, D], F8, kind="ExternalInput")
    WvT = nc.dram_tensor("WvT", [NDR, P, 2, D], F8, kind="ExternalInput")
    WoT = nc.dram_tensor("WoT", [D, D], BF16, kind="ExternalInput")
    bqv = nc.dram_tensor("bq", [D], F32, kind="ExternalInput")
    gv = nc.dram_tensor("ln_g", [D], F32, kind="ExternalInput")
    bv2 = nc.dram_tensor("ln_b", [D], F32, kind="ExternalInput")
    ones8 = nc.dram_tensor("onesv", [P, NRT_V * H], F8, kind="ExternalInput")
    onesf = nc.dram_tensor("onesf", [DK], F32, kind="ExternalInput")
    out = nc.dram_tensor("out", [M, D], F32, kind="ExternalOutput")

    WoT_r = WoT[:, :].rearrange("(a p) o -> p a o", p=P)

    with tile.TileContext(nc) as tc:
      for _rep in range(max(1, unroll)):
        pxo_cm = tc.tile_pool(name="pxo", bufs=1)
        pxo = pxo_cm.__enter__()
        with (
            tc.tile_pool(name="pqv", bufs=1) as pqv,
        ):
            XO = [
                pxo.tile([P, M], BF16, tag=f"XO{i}", name=f"XO{i}")
                for i in range(NHP)
            ]
            if "noattn" in ab:
                for t in XO:
                    nc.vector.memset(t, 0.001)
            exf = None
            if "noexp" in ab:
                exf = pxo.tile([P, 2, 512], F8, tag="exf", name="exf")
                nc.vector.memset(exf, 0.001)

            QT = []
            for ot in range(NOT):
                t = pqv.tile([P, M], BF16, tag=f"QT{ot}", name=f"QT{ot}")
                QT.append(t)
            bq_p = pqv.tile([P, NOT], F32)
            nc.gpsimd.dma_start(bq_p, bqv[:].rearrange("(a p) -> p a", p=P))
            ones_t = pqv.tile([1, DK], MM_DT)
            nc.gpsimd.dma_start(
                ones_t, onesf[:].partition_broadcast(1).bitcast(MM_DT)
            )
            Vp = []
            for rtp in range(NRT_V // 2):
                t = pqv.tile(
                    [P, 2, H, DK + 1], F8, tag=f"Vp{rtp}", name=f"Vp{rtp}"
                )
                nc.gpsimd.memset(t[:, :, :, DK : DK + 1], 1.0)
                Vp.append(t)
            # wv loads early so phase B starts without a DMA stall
            pwv_cm = tc.tile_pool(name="pwv", bufs=1)
            pwv = pwv_cm.__enter__()
            wv = []
            for g in range(NDR):
                w_t = pwv.tile([P, 2, D], F8, tag=f"wv{g}", name=f"wv{g}")
                nc.gpsimd.dma_start(w_t, WvT[g, :, :, :])
                wv.append(w_t)

            pbx_cm = tc.tile_pool(name="pbx", bufs=1)
            pbx = pbx_cm.__enter__()
            psX_cm = tc.tile_pool(name="psX", bufs=2, space="PSUM")
            psX = psX_cm.__enter__()

            # ---- Phase A: Q^T = (Wq/8) @ x_q^T + bq/8, layout [o, r]
            pa_cm = tc.tile_pool(name="pa", bufs=1)
            pa = pa_cm.__enter__()
            if True:
                wq = []
                xq = []
                for g in range(NDR):
                    w_t = pa.tile([P, 2, D], F8, tag=f"wq{g}", name=f"wq{g}")
                    nc.sync.dma_start(w_t, WqT[g, :, :, :])
                    wq.append(w_t)
                    x_t = pa.tile([P, 2, M], F8, tag=f"xq{g}", name=f"xq{g}")
                    nc.sync.dma_start(x_t, xqT[g, :, :, :])
                    xq.append(x_t)
                xv = []
                for g in range(NDR):
                    x_t = pbx.tile([P, 2, S], F8, tag=f"xv{g}", name=f"xv{g}")
                    nc.sync.dma_start(x_t, xvT[g, :, :, :])
                    xv.append(x_t)
                a_work = []

                def a_group(ot, qt):
                    ps = psX.tile([P, 512], F32, tag='px', name='px')
                    for g in range(NDR):
                        nc.tensor.matmul(
                            ps,
                            wq[g][:, :, ot * P : (ot + 1) * P],
                            xq[g][:, :, qt * 512 : (qt + 1) * 512],
                            start=(g == 0),
                            stop=(g == NDR - 1),
                            perf_mode=mybir.MatmulPerfMode.DoubleRow,
                        )
                    nc.vector.tensor_scalar_add(
                        QT[ot][:, qt * 512 : (qt + 1) * 512],
                        ps,
                        bq_p[:, ot : ot + 1],
                    )

                for qt in range(NQT):
                    a_group(0, qt)
                for ot in range(1, NOT):
                    for qt in range(NQT):
                        a_work.append(
                            lambda ot=ot, qt=qt: a_group(ot, qt)
                        )

            # xk/wk load during phase B so phase D starts without a DMA stall
            pdx_cm = tc.tile_pool(name="pdx", bufs=1)
            pdx = pdx_cm.__enter__()
            xk = []
            wk = []
            for g in range(NDR):
                x_t = pdx.tile([P, 2, S], F8, tag=f"xk{g}", name=f"xk{g}")
                nc.gpsimd.dma_start(x_t, xkT[g, :, :, :])
                xk.append(x_t)
            for g in range(NDR):
                w_t = pdx.tile([P, 2, D], F8, tag=f"wk{g}", name=f"wk{g}")
                nc.gpsimd.dma_start(w_t, WkT[g, :, :, :])
                wk.append(w_t)

            # ---- Phase B: V = x_v @ Wv^T (bias folded into qres), [r, o] fp8
            # pairs. First half runs as a phase (needed by the first PV
            # groups); second half is folded into the first attention section.
            b_work = []

            def b_group(rt, o2):
                ps = psX.tile([P, 512], F32, tag='px', name='px')
                for g in range(NDR):
                    nc.tensor.matmul(
                        ps,
                        xv[g][:, :, rt * P : (rt + 1) * P],
                        wv[g][:, :, o2 * 512 : (o2 + 1) * 512],
                        start=(g == 0),
                        stop=(g == NDR - 1),
                        perf_mode=mybir.MatmulPerfMode.DoubleRow,
                    )
                nc.vector.tensor_copy(
                    Vp[rt // 2][:, rt % 2, o2 * 8 : (o2 + 1) * 8, 0:DK],
                    ps[:, :].rearrange("p (h e) -> p h e", e=DK),
                )

            for rt in range(NRT_V // 2):
                for o2 in range(2):
                    b_group(rt, o2)
            for rt in range(NRT_V // 2, NRT_V):
                for o2 in range(2):
                    b_work.append(lambda rt=rt, o2=o2: b_group(rt, o2))

            # wo + E-state prefetch during D so phase E work can interleave
            pwo_cm = tc.tile_pool(name="pwo", bufs=NDT, side="right")
            pwo = pwo_cm.__enter__()
            wo = []
            for dt in range(NDT):
                w_t = pwo.tile([P, D], BF16, tag="wo", name=f"wo{dt}")
                nc.gpsimd.dma_start(w_t, WoT_r[:, dt, :])
                wo.append(w_t)
            pec_cm = tc.tile_pool(name="pec", side="right", bufs=1)
            pec = pec_cm.__enter__()
            peq_cm = tc.tile_pool(name="peq", side="right", bufs=1)
            peq = peq_cm.__enter__()
            pey_cm = tc.tile_pool(name="pey", side="right", bufs=4)
            pey = pey_cm.__enter__()
            pst_cm = tc.tile_pool(name="pst", side="right", bufs=8)
            pst = pst_cm.__enter__()
            g_b = pec.tile([P, D], F32)
            b_b = pec.tile([P, D], F32)
            eps_t = pec.tile([P, 1], F32)
            nc.sync.dma_start(g_b, gv[:].partition_broadcast(P))
            nc.sync.dma_start(b_b, bv2[:].partition_broadcast(P))
            nc.vector.memset(eps_t, 1e-5)
            qrs = []
            for rt in range(NRT_O):
                qr = peq.tile([P, D], BF16, tag=f"qr{rt}", name=f"qr{rt}")
                nc.gpsimd.dma_start(qr, qres[rt * P : (rt + 1) * P, :])
                qrs.append(qr)
            e_state = {}

            def e_mm_half(rt, o2, half, pspool):
                st = e_state.setdefault(rt, {})
                if "y" not in st:
                    st["y"] = pey.tile([P, D], F32, tag="y", name="y")
                if half == 0:
                    st[f"ps{o2}"] = pspool.tile(
                        [P, 512], F32, tag="px", name="eps"
                    )
                ps = st[f"ps{o2}"]
                for hp in range(half * 4, half * 4 + 4):
                    nc.tensor.matmul(
                        ps,
                        XO[hp][:, rt * P : (rt + 1) * P],
                        wo[hp][:, o2 * 512 : (o2 + 1) * 512],
                        start=(hp == 0),
                        stop=(hp == NOT - 1),
                    )
                if half == 1:
                    nc.vector.tensor_add(
                        st["y"][:, o2 * 512 : (o2 + 1) * 512],
                        ps,
                        qrs[rt][:, o2 * 512 : (o2 + 1) * 512],
                    )
                    del st[f"ps{o2}"]

            def e_mm(rt, o2, pspool):
                e_mm_half(rt, o2, 0, pspool)
                e_mm_half(rt, o2, 1, pspool)

            def e_stats(rt):
                st = e_state[rt]
                y = st["y"]
                stats = pst.tile([P, 2, 6], F32)
                for sg in range(2):
                    nc.vector.bn_stats(
                        stats[:, sg, :], y[:, sg * 512 : (sg + 1) * 512]
                    )
                mv = pst.tile([P, 2], F32)
                nc.vector.bn_aggr(mv, stats)
                std = pst.tile([P, 1], F32)
                nc.scalar.activation(std, mv[:, 1:2], AF.Sqrt, bias=eps_t)
                rstd = pst.tile([P, 1], F32)
                nc.vector.reciprocal(rstd, std)
                st["mv"] = mv
                st["rstd"] = rstd

            def e_final(rt):
                st = e_state.pop(rt)
                y = st["y"]
                eng = nc.vector if rt < NRT_O // 2 else nc.gpsimd
                eng.tensor_scalar(
                    y,
                    y,
                    st["mv"][:, 0:1],
                    st["rstd"],
                    op0=ALU.subtract,
                    op1=ALU.mult,
                )
                eng.tensor_mul(y, y, g_b)
                eng.tensor_add(y, y, b_b)
                nc.sync.dma_start(out[rt * P : (rt + 1) * P, :], y)

            # ---- Phase D: K^T projection fused with attention, per head pair.
            # Software-pipelined per (qt, h01) section of 8 KG=2 groups:
            # PE scores(g+1) || ACT exp(g) || PE pv(g-1). PV PSUM and score
            # PSUM are double-buffered so section boundaries don't stall, and
            # the normalize chain is deferred into the next section's slack.
            KG = 2
            NG = NKT // KG
            with (
                tc.tile_pool(name="pdkt", bufs=1) as pdkt,
                tc.tile_pool(name="pde", bufs=4) as pde,
                tc.tile_pool(name="pdr", bufs=2) as pdr,
                tc.tile_pool(name="psS", bufs=2, space="PSUM") as psS,
                tc.tile_pool(name="psPV", bufs=2, space="PSUM") as psPV,
            ):
                kts = {}
                pending = []
                secno = [0]

                def flush_pending(g=99, force=False):
                    cur = (secno[0], g)
                    while pending and (force or pending[0][0] <= cur):
                        pending.pop(0)[1]()

                def kproj_start(hp):
                    kts[hp] = pdkt.tile(
                        [P, S], F8, tag=f"kt{hp}", name=f"kt{hp}"
                    )

                def kproj_chunk(hp, rt):
                    kt_t = kts[hp]
                    ps = psX.tile([P, 512], F32, tag="px", name="kps")
                    for g in range(NDR):
                        nc.tensor.matmul(
                            ps,
                            wk[g][:, :, hp * P : (hp + 1) * P],
                            xk[g][:, :, rt * 512 : (rt + 1) * 512],
                            start=(g == 0),
                            stop=(g == NDR - 1),
                            perf_mode=mybir.MatmulPerfMode.DoubleRow,
                        )
                    nc.vector.tensor_copy(
                        kt_t[:, rt * 512 : (rt + 1) * 512], ps
                    )

                def section(hp, qt, h01, kp_slots):
                    kt_t = kts[hp]
                    head = 2 * hp + h01
                    pb_ = h01 * DK
                    qsl = slice(qt * 512, (qt + 1) * 512)
                    pv = psPV.tile([DK + 1, 512], F32, tag="pv", name="pv")
                    prev = None
                    for g in range(NG):
                        ss = psS.tile([P, KG, 512], F32, tag="ss", name="ss")
                        for j in range(KG):
                            kt = g * KG + j
                            nc.tensor.matmul(
                                ss[:, j, :],
                                kt_t[pb_ : pb_ + DK, kt * P : (kt + 1) * P],
                                QT[hp][pb_ : pb_ + DK, qsl],
                                start=True,
                                stop=True,
                                tile_position=(pb_, 0),
                            )
                        if g in (1, 4):
                            flush_pending(g)
                        if g in kp_slots:
                            kp_slots[g]()
                        if secno[0] == 0 and b_work:
                            b_work.pop(0)()
                            if b_work:
                                b_work.pop(0)()
                        elif secno[0] >= 1 and g in (2, 5) and a_work:
                            a_work.pop(0)()
                        if "noexp" in ab:
                            ex = exf
                        else:
                            ex = pde.tile(
                                [P, KG, 512], F8, tag="ex", name="ex"
                            )
                            nc.scalar.activation(ex, ss, AF.Exp)
                        if prev is not None:
                            pex, pg = prev
                            nc.tensor.matmul(
                                pv,
                                Vp[pg][:, :, head, :],
                                pex,
                                start=(pg == 0),
                                stop=False,
                                perf_mode=mybir.MatmulPerfMode.DoubleRow,
                            )
                        prev = (ex, g)
                    pex, pg = prev
                    nc.tensor.matmul(
                        pv,
                        Vp[pg][:, :, head, :],
                        pex,
                        start=False,
                        stop=(pg == NG - 1),
                        perf_mode=mybir.MatmulPerfMode.DoubleRow,
                    )
                    dst = XO[hp][pb_ : pb_ + DK, qsl]
                    if "nonorm" in ab:
                        nc.vector.tensor_copy(dst, pv[0:DK, :])
                        return
                    rc = pdr.tile([1, 512], MM_DT, tag="rc", name="rc")
                    with nc.allow_low_precision(
                        reason="1/denom feeds f32r broadcast matmul"
                    ):
                        nc.vector.reciprocal(rc, pv[DK : DK + 1, :])

                    def norm(pv=pv, rc=rc, dst=dst):
                        rbp = psX.tile([DK, 512], F32, tag="px", name="rbp")
                        nc.tensor.matmul(rbp, ones_t, rc, start=True, stop=True)
                        nc.vector.tensor_copy(dst, pv[0:DK, :])
                        nc.vector.tensor_mul(dst, dst, rbp)

                    pending.append(((secno[0] + 1, 4), norm))
                    secno[0] += 1

                if "noattn" not in ab:
                    for hp in range(NHP):
                        kproj_start(hp)
                    for rt in range(NRT_K):
                        kproj_chunk(0, rt)
                    for hp in range(NHP):
                        for h01 in range(2):
                            kp = {}
                            if hp + 1 < NHP:
                                for g, rt_ in ((3, h01 * 2), (6, h01 * 2 + 1)):
                                    kp[g] = (
                                        lambda hp=hp, rt=rt_: kproj_chunk(
                                            hp + 1, rt
                                        )
                                    )
                            section(hp, 0, h01, kp)
                    sec = 0
                    for hp in range(NHP):
                        for h01 in range(2):
                            rt, part = divmod(sec, 4)
                            if part in (0, 1):
                                slots = {
                                    3: lambda rt=rt, o2=part: e_mm_half(
                                        rt, o2, 0, psX
                                    ),
                                    6: lambda rt=rt, o2=part: e_mm_half(
                                        rt, o2, 1, psX
                                    ),
                                }
                            elif part == 2:
                                slots = {3: lambda rt=rt: e_stats(rt)}
                            else:
                                slots = {3: lambda rt=rt: e_final(rt)}
                            section(hp, 1, h01, slots)
                            sec += 1
                    flush_pending(force=True)

            pdx_cm.__exit__(None, None, None)
            pa_cm.__exit__(None, None, None)
            pbx_cm.__exit__(None, None, None)
            pwv_cm.__exit__(None, None, None)
            psX_cm.__exit__(None, None, None)

        # ---- Phase E tail: remaining output rows (rt 4..7)
        with (
            tc.tile_pool(name="psE", bufs=2, space="PSUM") as psE,
        ):
            start_rt = 0 if "noattn" in ab else NRT_O // 2
            for base in range(start_rt, NRT_O, 4):
                rts = range(base, min(base + 4, NRT_O))
                for rt in rts:
                    e_mm(rt, 0, psE)
                    e_mm(rt, 1, psE)
                for rt in rts:
                    e_stats(rt)
                for rt in rts:
                    e_final(rt)
        pst_cm.__exit__(None, None, None)
        pey_cm.__exit__(None, None, None)
        peq_cm.__exit__(None, None, None)
        pec_cm.__exit__(None, None, None)
        pwo_cm.__exit__(None, None, None)
        pxo_cm.__exit__(None, None, None)
    _split_sync_waits(nc)
    return nc


_NC = None


def _get_nc():
    global _NC
    if _NC is None:
        _NC = build_nc()
    return _NC


def _pack_dr(xT, dt):
    """[D, N] -> [D//256, 128, 2, N] DoubleRow-packed: feature g*256+j*128+p
    lands at [g, p, j]."""
    n = xT.shape[1]
    return np.ascontiguousarray(
        xT.reshape(D // 256, 2, P, n).transpose(0, 2, 1, 3).astype(dt)
    )


def prepare_in_maps(q, k, v, Wq, bq, Wk, bk, Wv, bv, Wo, bo, ln_g, ln_b):
    f = np.float32
    f8 = ml_dtypes.float8_e4m3
    q = np.asarray(q, f)
    k = np.asarray(k, f)
    v = np.asarray(v, f)
    scale = 1.0 / np.sqrt(np.float32(DK))
    WqT = _pack_dr(np.asarray(Wq, f).T * scale, f8)
    WkT = _pack_dr(np.asarray(Wk, f).T, f8)
    WvT = _pack_dr(np.asarray(Wv, f).T, f8)
    WoT = np.ascontiguousarray(np.asarray(Wo, f).T.astype(ml_dtypes.bfloat16))
    bq_s = np.asarray(bq, f) * scale
    # bv flows through attention unchanged (probs sum to 1), so its effect on
    # the O projection is the constant vector bv @ Wo^T — fold into qres.
    res_const = np.asarray(bo, f) + np.asarray(bv, f) @ np.asarray(Wo, f).T
    common = {
        "WqT": WqT,
        "WkT": WkT,
        "WvT": WvT,
        "WoT": WoT,
        "bq": bq_s,
        "ln_g": np.asarray(ln_g, f),
        "ln_b": np.asarray(ln_b, f),
        "onesv": np.ones((P, NRT_V * H), f8),
        "onesf": np.ones(DK, np.float32),
    }
    in_maps = []
    for c in range(8):
        b_, half = divmod(c, 2)
        qs = q[b_, half * M : (half + 1) * M, :]
        qres_c = qs + res_const[None, :]
        in_maps.append(
            dict(
                common,
                xqT=_pack_dr(np.ascontiguousarray(qs.T), f8),
                xkT=_pack_dr(np.ascontiguousarray(k[b_].T), f8),
                xvT=_pack_dr(np.ascontiguousarray(v[b_].T), f8),
                qres=np.ascontiguousarray(qres_c.astype(ml_dtypes.bfloat16)),
            )
        )
    return in_maps


def kernel(q, k, v, Wq, bq, Wk, bk, Wv, bv, Wo, bo, ln_g, ln_b):
    nc = _get_nc()
    in_maps = prepare_in_maps(q, k, v, Wq, bq, Wk, bk, Wv, bv, Wo, bo, ln_g, ln_b)
    res = run_bass_kernel_spmd(nc, in_maps, core_ids=list(range(8)))
    out = np.empty((B, S, D), np.float32)
    for c in range(8):
        b_, half = divmod(c, 2)
        out[b_, half * M : (half + 1) * M, :] = res.results[c]["out"]
    return out
